# revision 20
# baseline (speedup 1.0000x reference)
"""Trainium2 Bass kernel for GCFAgg-style block:
    q1 = x@W1.T+b1; q2 = x@W2.T+b2; r = x@WR.T+br
    out = (q1 @ q2.T) @ r        (per batch, no softmax)

Key algebraic restructuring: with x_aug = [x | 1] and W*_aug = [W* | b*],
    out = x_aug @ (Khat @ (x_aug.T @ x_aug) @ Rhat)
where Khat = W1_aug.T @ W2_aug and Rhat = WR_aug.T are tiny host-precomputed
matrices. The device computes G = x.T @ x (symmetric: upper block-triangle
on PE, lower from PE transposes), the small chain P = Khat @ G @ Rhat, and
the projection out.T = P[:512].T @ x.T + v.

Work placement:
  - v (= P_aug row 512) is host-computed in O(N*D).
  - The rank-1 augmented terms of the chain (sx (x) rhat_row512 and
    khat_col512 (x) m1row) are materialized by the Scalar engine during the
    G window (scale-by-per-partition-scalar of a host-broadcast row) and
    folded into the chain's PSUM->SBUF copies as DVE adds — no K=1 PE
    matmuls.
  - G-symmetry: M1 groups run in order g1=3..0; group 3 needs only upper
    blocks, and each PE transpose that fills a lower block is interleaved
    right before the first group that consumes it.
  - out.T orientation makes +v a per-partition bias fused into the Scalar
    engine PSUM->SBUF copy; host reassembles the transposed output.

Perf notes (per core, PE @2.4GHz, ~332GB/s HBM):
  - PE ~127k cycles = 53us is the floor (G 41k, chain ~18k, out 65.5k).
  - bf16 x streams and P buy DMA bytes (bf16/f32r matmul are both
    1 cycle/row); the chain stays f32r for accuracy.
  - DMA triggers cost ~600ns of queue-engine time each regardless of size,
    so tiles move in large batched triggers; xa prefetch depth (pool bufs)
    covers all 10 triggers so the stream never backpressures.
  - A few warmup matmuls run during the initial DMA wait to pre-ramp the
    PE clock (0.65/1.2 GHz p-states before 3us of continuous work).

Sharding: batch dim B=8, one batch per NeuronCore (data parallel).

Self-contained: hardcodes shapes from the problem spec
(x: [8, 4096, 512] f32; W*: [512, 512]; b*: [512]).
"""
import os
import sys

sys.path.insert(0, "/opt/trn_rl_repo")

import numpy as np
import ml_dtypes

import concourse.bass as bass
import concourse.mybir as mybir
import concourse.tile as tile
from concourse import bacc
from concourse.bass_utils import run_bass_kernel_spmd
from concourse.masks import make_identity
from concourse.tile_rust import add_dep_helper

B = 8          # batch -> one per core
N = 4096       # tokens per batch
D = 512        # model dim
NT = N // 128  # 32 row tiles
NSLAB = 8      # 512-token slabs for the out.T phase
N_CORES = 8

F32 = mybir.dt.float32
F32R = mybir.dt.float32r
BF16 = mybir.dt.bfloat16

# mode: "bf16" (bf16 x/P storage+matmul, f32r chain) or "f32r"
MODE = os.environ.get("GCF_MODE", "bf16")

# xa trigger batching: first single tile goes on gpsimd (earliest-ready
# queue), the rest stream on sync
XA_BATCHES = [1, 1, 2, 4, 4, 4, 4, 4, 4, 4]
N_WARM = 10

_built = {}


def _build(mode):
    if mode in _built:
        return _built[mode]

    big = BF16 if mode == "bf16" else F32R
    chain = F32R

    def mm_ap(ap, dt):
        return ap if ap.dtype == dt else ap.bitcast(dt)

    nc = bacc.Bacc("TRN2", target_bir_lowering=False, debug=False,
                   num_devices=N_CORES)

    # all DRAM tensors are laid out partition-dim first by the host
    xa_d = nc.dram_tensor("xa", (128, NT, D), big, kind="ExternalInput")
    xat_d = nc.dram_tensor("xat", (4, 128, NSLAB, D), big, kind="ExternalInput")
    khat_d = nc.dram_tensor("khat", (128, 4, D), chain, kind="ExternalInput")
    rhat_d = nc.dram_tensor("rhat", (128, 4, D), chain, kind="ExternalInput")
    # scl[:, 0:4] = v (bias), [:, 4:8] = sx columns, [:, 8:12] = khat col 512
    scl_d = nc.dram_tensor("scl", (128, 12), F32, kind="ExternalInput")
    # broadcast rows: [0] = Rhat row 512, [1] = m1row (both repl. 128x)
    brow_d = nc.dram_tensor("brow", (128, 2, D), F32, kind="ExternalInput")
    out_d = nc.dram_tensor("out", (4, 128, NSLAB, D), BF16,
                           kind="ExternalOutput")

    with tile.TileContext(nc) as tc:
        with (
            tc.tile_pool(name="xa", bufs=len(XA_BATCHES)) as xa_pool,
            tc.tile_pool(name="const", bufs=1) as const_pool,
            tc.tile_pool(name="gsb", bufs=1) as g_pool,
            tc.tile_pool(name="chain", bufs=1) as chain_pool,
            tc.tile_pool(name="outsb", bufs=3) as out_pool,
        ):
            # ---- xa stream: batch 0 on gpsimd (its preamble ends first) ----
            xa_tiles = []          # (tile, sub-index) per global row tile
            t0 = 0
            for bi, nb in enumerate(XA_BATCHES):
                xa_t = xa_pool.tile([128, 4, D], big, tag="xa")
                eng = nc.gpsimd if bi == 0 else nc.sync
                eng.dma_start(xa_t[:, :nb, :], xa_d.ap()[:, t0:t0 + nb, :])
                for j in range(nb):
                    xa_tiles.append((xa_t, j))
                t0 += nb

            ident = const_pool.tile([128, 128], F32, tag="ident")
            make_identity(nc, ident[:])
            warm_sb = const_pool.tile([128, D], big, tag="warm")
            nc.vector.memset(warm_sb[:], 0.0)

            khat_sb = const_pool.tile([128, 4, D], chain, tag="khat")
            rhat_sb = const_pool.tile([128, 4, D], chain, tag="rhat")
            scl_sb = const_pool.tile([128, 12], F32, tag="scl")
            brow_sb = const_pool.tile([128, 2, D], F32, tag="brow")

            # ---- phase 1: G = x^T @ x; upper block-triangle only ----
            g_sb = [g_pool.tile([128, D], chain, tag=f"g{c}", name=f"g{c}")
                    for c in range(4)]
            with tc.tile_pool(name="psG", bufs=1, space="PSUM") as psG_pool:
                # warmup matmuls: pre-ramp the PE clock while the first xa
                # tiles are still in flight (results unused)
                ps_w = psG_pool.tile([128, D], F32, tag="warm")
                for _ in range(N_WARM):
                    nc.tensor.matmul(ps_w[:], mm_ap(warm_sb[:, :128], big),
                                     mm_ap(warm_sb[:], big),
                                     start=True, stop=True)

                ps_ga = [psG_pool.tile([128, D - c * 128], F32, tag=f"ga{c}",
                                       name=f"ga{c}") for c in range(4)]
                gate_mms = []
                for t in range(NT):
                    xa_t, j = xa_tiles[t]
                    for c in range(4):
                        mm = nc.tensor.matmul(
                            ps_ga[c][:],
                            mm_ap(xa_t[:, j, c * 128:(c + 1) * 128], big),
                            mm_ap(xa_t[:, j, c * 128:D], big),
                            start=(t == 0), stop=(t == NT - 1),
                        )
                        if c == 3:
                            gate_mms.append(mm)

                # consts on gpsimd, gated by first use so they never crowd
                # the xa stream: rhat feeds the first M1 group right at G
                # end; khat is first needed ~6us later at the P groups;
                # scl/brow at the chain adds
                cd = nc.gpsimd.dma_start(rhat_sb[:], rhat_d.ap()[:])
                add_dep_helper(cd.ins, gate_mms[8].ins,
                               reason="rhat gated behind G t=8")
                cd = nc.gpsimd.dma_start(khat_sb[:], khat_d.ap()[:])
                add_dep_helper(cd.ins, gate_mms[20].ins,
                               reason="khat gated behind G t=20")
                for cd in (nc.gpsimd.dma_start(scl_sb[:], scl_d.ap()[:]),
                           nc.gpsimd.dma_start(brow_sb[:], brow_d.ap()[:])):
                    add_dep_helper(cd.ins, gate_mms[24].ins,
                                   reason="scl/brow gated behind G t=24")

                # xat loads (sync queue, after the xa triggers in program
                # order) gated behind the G tail: during G the xa stream +
                # consts saturate HBM; the chain window is otherwise idle.
                xat_sb = [const_pool.tile([128, NSLAB, D], big, tag=f"xat{c}",
                                          name=f"xat{c}") for c in range(4)]
                for h in range(2):
                    for c in range(4):
                        xd = nc.sync.dma_start(
                            xat_sb[c][:, 4 * h:4 * h + 4, :],
                            xat_d.ap()[c][:, 4 * h:4 * h + 4, :])
                        add_dep_helper(xd.ins, gate_mms[26 if h == 0 else 31].ins,
                                       reason="xat gated behind G tail")

                # ---- phase 2 interleaved with G wrap-up; the upper-block
                # PSUM->SBUF copies are emitted just-in-time per column so
                # the DVE backlog never stalls the next M1 group ----
                # the copies run on the Scalar engine (idle until phase 3,
                # and it CAN read PSUM) so the DVE queue holds only the
                # chain adds — neither engine's backlog stalls the PE
                def copy_col(g1):
                    for c in range(g1 + 1):
                        nc.scalar.copy(
                            g_sb[c][:, g1 * 128:(g1 + 1) * 128],
                            ps_ga[c][:, (g1 - c) * 128:(g1 - c + 1) * 128])

                with tc.tile_pool(name="psC", bufs=2, space="PSUM") as psC_pool:
                    m1_sb = [chain_pool.tile([128, D], chain, tag=f"m1{c}",
                                             name=f"m1{c}") for c in range(4)]

                    def transpose_block(c1, c2):
                        # fill lower block (c2, c1) from upper (c1, c2)
                        ps_tr = psC_pool.tile([128, 128], F32, tag="tr", bufs=1)
                        nc.tensor.transpose(
                            ps_tr[:],
                            mm_ap(g_sb[c1][:, c2 * 128:(c2 + 1) * 128], F32),
                            ident[:],
                        )
                        nc.scalar.copy(
                            g_sb[c2][:, c1 * 128:(c1 + 1) * 128], ps_tr[:])

                    # M1 groups g1 = 3..0; PE transposes that fill the lower
                    # blocks needed by group g1 are emitted just before it
                    for g1 in range(3, -1, -1):
                        copy_col(g1)
                        if g1 == 2:
                            transpose_block(2, 3)
                        elif g1 == 1:
                            transpose_block(1, 2)
                            transpose_block(1, 3)
                        elif g1 == 0:
                            transpose_block(0, 1)
                            transpose_block(0, 2)
                            transpose_block(0, 3)
                        ps = psC_pool.tile([128, D], F32, tag="chain", bufs=2)
                        for i, g2 in enumerate(
                                list(range(g1 + 1)) + list(range(g1 + 1, 4))):
                            nc.tensor.matmul(
                                ps[:],
                                mm_ap(g_sb[g2][:, g1 * 128:(g1 + 1) * 128],
                                      chain),
                                mm_ap(rhat_sb[:, g2, :], chain),
                                start=(i == 0), stop=(i == 3),
                            )
                        # m1 = ps + sx[g1-block] (x) Rhat[512,:] — the rank-1
                        # augmented term folds into the PSUM->SBUF copy
                        nc.vector.scalar_tensor_tensor(
                            m1_sb[g1][:], brow_sb[:, 0, :],
                            scl_sb[:, 4 + g1:5 + g1], ps[:],
                            mybir.AluOpType.mult, mybir.AluOpType.add)

                    p_sb = [chain_pool.tile([128, D], big, tag=f"p{c}",
                                            name=f"p{c}") for c in range(4)]
                    for g1 in range(4):
                        ps = psC_pool.tile([128, D], F32, tag="chain", bufs=2)
                        for g2 in range(4):
                            nc.tensor.matmul(
                                ps[:],
                                mm_ap(khat_sb[:, g2, g1 * 128:(g1 + 1) * 128],
                                      chain),
                                mm_ap(m1_sb[g2][:], chain),
                                start=(g2 == 0), stop=(g2 == 3),
                            )
                        # p = ps + Khat[g1-block, 512] (x) m1row
                        nc.vector.scalar_tensor_tensor(
                            p_sb[g1][:], brow_sb[:, 1, :],
                            scl_sb[:, 8 + g1:9 + g1], ps[:],
                            mybir.AluOpType.mult, mybir.AluOpType.add)

            # ---- phase 3: out.T[db,:] = sum_c P[c,db].T @ x.T[c,:] + v[db]
            # (+v fused into the Scalar-engine PSUM->SBUF copy as a
            # per-partition bias) ----
            with tc.tile_pool(name="psO", bufs=1, space="PSUM") as psO_pool:
                nst = 0
                for h in range(2):
                    for db in range(4):
                        ot = out_pool.tile([128, 4, D], BF16, tag="ot")
                        for si in range(4):
                            s = 4 * h + si
                            ps = psO_pool.tile([128, D], F32, tag="out", bufs=6)
                            for c in range(4):
                                nc.tensor.matmul(
                                    ps[:],
                                    mm_ap(p_sb[c][:, db * 128:(db + 1) * 128],
                                          big),
                                    mm_ap(xat_sb[c][:, s, :], big),
                                    start=(c == 0), stop=(c == 3),
                                )
                            nc.scalar.add(ot[:, si, :], ps[:],
                                          scl_sb[:, db:db + 1])
                            # split the final buffer's store in two so the
                            # last transfer off the critical path is short
                            if h == 1 and db == 3 and si in (1, 3):
                                eng = nc.gpsimd if nst % 2 == 0 else nc.sync
                                nst += 1
                                lo = 4 * h + (0 if si == 1 else 2)
                                eng.dma_start(
                                    out_d.ap()[db][:, lo:lo + 2, :],
                                    ot[:, (0 if si == 1 else 2):
                                       (2 if si == 1 else 4), :])
                        if not (h == 1 and db == 3):
                            eng = nc.gpsimd if nst % 2 == 0 else nc.sync
                            nst += 1
                            eng.dma_start(
                                out_d.ap()[db][:, 4 * h:4 * h + 4, :], ot[:])

    nc.compile()
    _built[mode] = nc
    return nc


def _prep_host(x, Wq1_w, Wq1_b, Wq2_w, Wq2_b, WR_w, WR_b, mode):
    f, f8 = np.float32, np.float64
    W1a = np.concatenate([Wq1_w, Wq1_b[:, None]], axis=1)   # [512, 513]
    W2a = np.concatenate([Wq2_w, Wq2_b[:, None]], axis=1)
    WRa = np.concatenate([WR_w, WR_b[:, None]], axis=1)

    khatT = (W2a.T.astype(f8) @ W1a.astype(f8)).astype(f)   # [513, 513]
    rhat = WRa.T.astype(f)                                  # [513, 512]
    khat2 = np.ascontiguousarray(
        khatT[:D, :D].reshape(4, 128, D).transpose(1, 0, 2))
    rhat2 = np.ascontiguousarray(
        rhat[:D].reshape(4, 128, D).transpose(1, 0, 2))

    sx = x.sum(axis=1, dtype=f8).astype(f)                  # [B, 512]
    sxa = np.concatenate([sx, np.full((B, 1), float(N), f)], axis=1)
    m1row = (sxa.astype(f8) @ rhat.astype(f8)).astype(f)    # [B, 512]

    # v = P_aug[512,:] = Khat_aug[512,:] @ G_aug @ Rhat, host-computable in
    # O(N*D): z = x@k[:512] + k[512];  v = [x.T z | sum z] @ Rhat
    k = (W1a[:, D].astype(f8) @ W2a.astype(f8))             # [513]
    z = x.astype(f8) @ k[:D] + k[D]                         # [B, 4096]
    u = np.concatenate([np.einsum('bn,bnd->bd', z, x.astype(f8)),
                        z.sum(axis=1)[:, None]], axis=1)    # [B, 513]
    v = (u @ rhat.astype(f8)).astype(f)                     # [B, 512]

    # scl[:, 0:4] = v, [:, 4:8] = sx, [:, 8:12] = Khat[:, 512], col-major
    scl = np.concatenate([
        v.reshape(B, 4, 128), sx.reshape(B, 4, 128),
        np.broadcast_to(khatT[D, :D].reshape(1, 4, 128), (B, 4, 128)),
    ], axis=1).transpose(0, 2, 1).astype(f)                 # [B, 128, 12]
    scl = np.ascontiguousarray(scl)
    brow = np.stack([
        np.broadcast_to(rhat[D], (B, 128, D)),
        np.repeat(m1row[:, None, :], 128, axis=1),
    ], axis=2).astype(f)                                    # [B, 128, 2, D]
    brow = np.ascontiguousarray(brow)

    dt = ml_dtypes.bfloat16 if mode == "bf16" else f
    xa2 = np.ascontiguousarray(
        x.reshape(B, NT, 128, D).transpose(0, 2, 1, 3)).astype(dt)
    xat2 = np.ascontiguousarray(
        x.transpose(0, 2, 1).reshape(B, 4, 128, NSLAB, D)).astype(dt)

    return [
        {"xa": xa2[b], "xat": xat2[b], "khat": khat2, "rhat": rhat2,
         "scl": scl[b], "brow": brow[b]}
        for b in range(B)
    ]


def _post(res):
    # out.T tiles [db, p, slab, j] -> out[slab*512+j, db*128+p]
    return np.stack([
        np.ascontiguousarray(
            res.results[b]["out"].astype(np.float32)
            .transpose(2, 3, 0, 1)).reshape(N, D)
        for b in range(B)
    ])


def kernel(x, Wq1_w, Wq1_b, Wq2_w, Wq2_b, WR_w, WR_b):
    x = np.asarray(x, dtype=np.float32)
    args = [np.asarray(a, dtype=np.float32)
            for a in (Wq1_w, Wq1_b, Wq2_w, Wq2_b, WR_w, WR_b)]
    in_maps = _prep_host(x, *args, MODE)

    nc = _build(MODE)
    # the axon-tunneled device occasionally starts in a wedged state
    # (NRT_EXEC_UNIT_UNRECOVERABLE) and recovers on the next attempt
    last_err = None
    for attempt in range(3):
        try:
            res = run_bass_kernel_spmd(nc, in_maps, core_ids=list(range(N_CORES)))
            break
        except Exception as e:  # noqa: BLE001
            last_err = e
            import time as _time
            _time.sleep(2.0)
            try:
                import jax
                jax.clear_caches()
            except Exception:
                pass
    else:
        raise last_err
    return _post(res)


# revision 21
# speedup vs baseline: 1.1089x; 1.1089x over previous
"""Trainium2 Bass kernel for GCFAgg-style block:
    q1 = x@W1.T+b1; q2 = x@W2.T+b2; r = x@WR.T+br
    out = (q1 @ q2.T) @ r        (per batch, no softmax)

Key algebraic restructuring: with x_aug = [x | 1] and W*_aug = [W* | b*],
    out = x_aug @ (Khat @ (x_aug.T @ x_aug) @ Rhat)
where Khat = W1_aug.T @ W2_aug and Rhat = WR_aug.T are tiny host-precomputed
matrices. The device computes G = x.T @ x (symmetric: upper block-triangle
on PE, lower from PE transposes), the small chain P = Khat @ G @ Rhat, and
the projection out.T = P[:512].T @ x.T + v.

Work placement:
  - v (= P_aug row 512) is host-computed in O(N*D).
  - The rank-1 augmented terms of the chain (sx (x) rhat_row512 and
    khat_col512 (x) m1row) are materialized by the Scalar engine during the
    G window (scale-by-per-partition-scalar of a host-broadcast row) and
    folded into the chain's PSUM->SBUF copies as DVE adds — no K=1 PE
    matmuls.
  - G-symmetry: M1 groups run in order g1=3..0; group 3 needs only upper
    blocks, and each PE transpose that fills a lower block is interleaved
    right before the first group that consumes it.
  - out.T orientation makes +v a per-partition bias fused into the Scalar
    engine PSUM->SBUF copy; host reassembles the transposed output.

Perf notes (per core, PE @2.4GHz, ~332GB/s HBM):
  - PE ~127k cycles = 53us is the floor (G 41k, chain ~18k, out 65.5k).
  - bf16 x streams and P buy DMA bytes (bf16/f32r matmul are both
    1 cycle/row); the chain stays f32r for accuracy.
  - DMA triggers cost ~600ns of queue-engine time each regardless of size,
    so tiles move in large batched triggers; xa prefetch depth (pool bufs)
    covers all 10 triggers so the stream never backpressures.
  - A few warmup matmuls run during the initial DMA wait to pre-ramp the
    PE clock (0.65/1.2 GHz p-states before 3us of continuous work).

Sharding: batch dim B=8, one batch per NeuronCore (data parallel).

Self-contained: hardcodes shapes from the problem spec
(x: [8, 4096, 512] f32; W*: [512, 512]; b*: [512]).
"""
import os
import sys

sys.path.insert(0, "/opt/trn_rl_repo")

import numpy as np
import ml_dtypes

import concourse.bass as bass
import concourse.mybir as mybir
import concourse.tile as tile
from concourse import bacc
from concourse.bass_utils import run_bass_kernel_spmd
from concourse.masks import make_identity
from concourse.tile_rust import add_dep_helper

B = 8          # batch -> one per core
N = 4096       # tokens per batch
D = 512        # model dim
NT = N // 128  # 32 row tiles
NSLAB = 8      # 512-token slabs for the out.T phase
N_CORES = 8

F32 = mybir.dt.float32
F32R = mybir.dt.float32r
BF16 = mybir.dt.bfloat16

# mode: "bf16" (bf16 x/P storage+matmul, f32r chain) or "f32r"
MODE = os.environ.get("GCF_MODE", "bf16")

# xa trigger batching: first single tile goes on gpsimd (earliest-ready
# queue), the rest stream on sync
XA_BATCHES = [1, 1, 2, 4, 4, 4, 4, 4, 4, 4]
N_WARM = 10

_built = {}


def _build(mode):
    if mode in _built:
        return _built[mode]

    big = BF16 if mode == "bf16" else F32R
    chain = F32R

    def mm_ap(ap, dt):
        return ap if ap.dtype == dt else ap.bitcast(dt)

    nc = bacc.Bacc("TRN2", target_bir_lowering=False, debug=False,
                   num_devices=N_CORES)

    # all DRAM tensors are laid out partition-dim first by the host
    xa_d = nc.dram_tensor("xa", (128, NT, D), big, kind="ExternalInput")
    xat_d = nc.dram_tensor("xat", (4, 128, NSLAB, D), big, kind="ExternalInput")
    khat_d = nc.dram_tensor("khat", (128, 4, D), chain, kind="ExternalInput")
    rhat_d = nc.dram_tensor("rhat", (128, 4, D), chain, kind="ExternalInput")
    # scl[:, 0:4] = v (bias), [:, 4:8] = sx columns, [:, 8:12] = khat col 512
    scl_d = nc.dram_tensor("scl", (128, 12), F32, kind="ExternalInput")
    # broadcast rows: [0] = Rhat row 512, [1] = m1row (both repl. 128x)
    brow_d = nc.dram_tensor("brow", (128, 2, D), F32, kind="ExternalInput")
    out_d = nc.dram_tensor("out", (4, 128, NSLAB, D), BF16,
                           kind="ExternalOutput")

    with tile.TileContext(nc) as tc:
        with (
            tc.tile_pool(name="xa", bufs=len(XA_BATCHES)) as xa_pool,
            tc.tile_pool(name="const", bufs=1) as const_pool,
            tc.tile_pool(name="gsb", bufs=1) as g_pool,
            tc.tile_pool(name="chain", bufs=1) as chain_pool,
            tc.tile_pool(name="outsb", bufs=3) as out_pool,
        ):
            # ---- xa stream: batch 0 on gpsimd (its preamble ends first) ----
            xa_tiles = []          # (tile, sub-index) per global row tile
            t0 = 0
            for bi, nb in enumerate(XA_BATCHES):
                xa_t = xa_pool.tile([128, 4, D], big, tag="xa")
                eng = nc.gpsimd if bi == 0 else nc.sync
                eng.dma_start(xa_t[:, :nb, :], xa_d.ap()[:, t0:t0 + nb, :])
                for j in range(nb):
                    xa_tiles.append((xa_t, j))
                t0 += nb

            ident = const_pool.tile([128, 128], F32, tag="ident")
            make_identity(nc, ident[:])
            warm_sb = const_pool.tile([128, D], big, tag="warm")
            nc.vector.memset(warm_sb[:], 0.0)

            khat_sb = const_pool.tile([128, 4, D], chain, tag="khat")
            rhat_sb = const_pool.tile([128, 4, D], chain, tag="rhat")
            scl_sb = const_pool.tile([128, 12], F32, tag="scl")
            brow_sb = const_pool.tile([128, 2, D], F32, tag="brow")

            # ---- phase 1: G = x^T @ x; upper block-triangle only ----
            g_sb = [g_pool.tile([128, D], chain, tag=f"g{c}", name=f"g{c}")
                    for c in range(4)]
            with tc.tile_pool(name="psG", bufs=1, space="PSUM") as psG_pool:
                # warmup matmuls: pre-ramp the PE clock while the first xa
                # tiles are still in flight (results unused)
                ps_w = psG_pool.tile([128, D], F32, tag="warm")
                for _ in range(N_WARM):
                    nc.tensor.matmul(ps_w[:], mm_ap(warm_sb[:, :128], big),
                                     mm_ap(warm_sb[:], big),
                                     start=True, stop=True)

                ps_ga = [psG_pool.tile([128, D - c * 128], F32, tag=f"ga{c}",
                                       name=f"ga{c}") for c in range(4)]
                gate_mms = []
                for t in range(NT):
                    xa_t, j = xa_tiles[t]
                    for c in range(4):
                        mm = nc.tensor.matmul(
                            ps_ga[c][:],
                            mm_ap(xa_t[:, j, c * 128:(c + 1) * 128], big),
                            mm_ap(xa_t[:, j, c * 128:D], big),
                            start=(t == 0), stop=(t == NT - 1),
                        )
                        if c == 3:
                            gate_mms.append(mm)

                # consts on gpsimd, gated by first use so they never crowd
                # the xa stream: rhat feeds the first M1 group right at G
                # end; khat is first needed ~6us later at the P groups;
                # scl/brow at the chain adds
                for cd in (nc.gpsimd.dma_start(scl_sb[:], scl_d.ap()[:]),
                           nc.gpsimd.dma_start(brow_sb[:], brow_d.ap()[:])):
                    add_dep_helper(cd.ins, gate_mms[8].ins,
                                   reason="scl/brow (tiny, needed first) t=8")
                cd = nc.gpsimd.dma_start(rhat_sb[:], rhat_d.ap()[:])
                add_dep_helper(cd.ins, gate_mms[10].ins,
                               reason="rhat gated behind G t=10")
                cd = nc.gpsimd.dma_start(khat_sb[:], khat_d.ap()[:])
                add_dep_helper(cd.ins, gate_mms[20].ins,
                               reason="khat gated behind G t=20")

                # xat loads (sync queue, after the xa triggers in program
                # order) gated behind the G tail: during G the xa stream +
                # consts saturate HBM; the chain window is otherwise idle.
                xat_sb = [const_pool.tile([128, NSLAB, D], big, tag=f"xat{c}",
                                          name=f"xat{c}") for c in range(4)]
                for h in range(2):
                    for c in range(4):
                        xd = nc.sync.dma_start(
                            xat_sb[c][:, 4 * h:4 * h + 4, :],
                            xat_d.ap()[c][:, 4 * h:4 * h + 4, :])
                        add_dep_helper(xd.ins, gate_mms[26 if h == 0 else 31].ins,
                                       reason="xat gated behind G tail")

                # ---- phase 2 interleaved with G wrap-up; the upper-block
                # PSUM->SBUF copies are emitted just-in-time per column so
                # the DVE backlog never stalls the next M1 group ----
                # the copies run on the Scalar engine (idle until phase 3,
                # and it CAN read PSUM) so the DVE queue holds only the
                # chain adds — neither engine's backlog stalls the PE
                def copy_col(g1):
                    for c in range(g1 + 1):
                        nc.scalar.copy(
                            g_sb[c][:, g1 * 128:(g1 + 1) * 128],
                            ps_ga[c][:, (g1 - c) * 128:(g1 - c + 1) * 128])

                with tc.tile_pool(name="psC", bufs=2, space="PSUM") as psC_pool:
                    m1_sb = [chain_pool.tile([128, D], chain, tag=f"m1{c}",
                                             name=f"m1{c}") for c in range(4)]

                    def transpose_block(c1, c2):
                        # fill lower block (c2, c1) from upper (c1, c2)
                        ps_tr = psC_pool.tile([128, 128], F32, tag="tr", bufs=1)
                        nc.tensor.transpose(
                            ps_tr[:],
                            mm_ap(g_sb[c1][:, c2 * 128:(c2 + 1) * 128], F32),
                            ident[:],
                        )
                        nc.scalar.copy(
                            g_sb[c2][:, c1 * 128:(c1 + 1) * 128], ps_tr[:])

                    # M1 groups g1 = 3..0; PE transposes that fill the lower
                    # blocks needed by group g1 are emitted just before it
                    for g1 in range(3, -1, -1):
                        copy_col(g1)
                        if g1 == 2:
                            transpose_block(2, 3)
                        elif g1 == 1:
                            transpose_block(1, 2)
                            transpose_block(1, 3)
                        elif g1 == 0:
                            transpose_block(0, 1)
                            transpose_block(0, 2)
                            transpose_block(0, 3)
                        ps = psC_pool.tile([128, D], F32, tag="chain", bufs=2)
                        for i, g2 in enumerate(
                                list(range(g1 + 1)) + list(range(g1 + 1, 4))):
                            nc.tensor.matmul(
                                ps[:],
                                mm_ap(g_sb[g2][:, g1 * 128:(g1 + 1) * 128],
                                      chain),
                                mm_ap(rhat_sb[:, g2, :], chain),
                                start=(i == 0), stop=(i == 3),
                            )
                        # m1 = ps + sx[g1-block] (x) Rhat[512,:] — the rank-1
                        # augmented term folds into the PSUM->SBUF copy
                        nc.vector.scalar_tensor_tensor(
                            m1_sb[g1][:], brow_sb[:, 0, :],
                            scl_sb[:, 4 + g1:5 + g1], ps[:],
                            mybir.AluOpType.mult, mybir.AluOpType.add)

                    p_sb = [chain_pool.tile([128, D], big, tag=f"p{c}",
                                            name=f"p{c}") for c in range(4)]
                    for g1 in range(4):
                        ps = psC_pool.tile([128, D], F32, tag="chain", bufs=2)
                        for g2 in range(4):
                            nc.tensor.matmul(
                                ps[:],
                                mm_ap(khat_sb[:, g2, g1 * 128:(g1 + 1) * 128],
                                      chain),
                                mm_ap(m1_sb[g2][:], chain),
                                start=(g2 == 0), stop=(g2 == 3),
                            )
                        # p = ps + Khat[g1-block, 512] (x) m1row
                        nc.vector.scalar_tensor_tensor(
                            p_sb[g1][:], brow_sb[:, 1, :],
                            scl_sb[:, 8 + g1:9 + g1], ps[:],
                            mybir.AluOpType.mult, mybir.AluOpType.add)

            # ---- phase 3: out.T[db,:] = sum_c P[c,db].T @ x.T[c,:] + v[db]
            # (+v fused into the Scalar-engine PSUM->SBUF copy as a
            # per-partition bias) ----
            with tc.tile_pool(name="psO", bufs=1, space="PSUM") as psO_pool:
                nst = 0
                for h in range(2):
                    for db in range(4):
                        ot = out_pool.tile([128, 4, D], BF16, tag="ot")
                        for si in range(4):
                            s = 4 * h + si
                            ps = psO_pool.tile([128, D], F32, tag="out", bufs=6)
                            for c in range(4):
                                nc.tensor.matmul(
                                    ps[:],
                                    mm_ap(p_sb[c][:, db * 128:(db + 1) * 128],
                                          big),
                                    mm_ap(xat_sb[c][:, s, :], big),
                                    start=(c == 0), stop=(c == 3),
                                )
                            nc.scalar.add(ot[:, si, :], ps[:],
                                          scl_sb[:, db:db + 1])
                            # split the final buffer's store in two so the
                            # last transfer off the critical path is short
                            if h == 1 and db == 3 and si in (1, 3):
                                eng = nc.gpsimd if nst % 2 == 0 else nc.sync
                                nst += 1
                                lo = 4 * h + (0 if si == 1 else 2)
                                eng.dma_start(
                                    out_d.ap()[db][:, lo:lo + 2, :],
                                    ot[:, (0 if si == 1 else 2):
                                       (2 if si == 1 else 4), :])
                        if not (h == 1 and db == 3):
                            eng = nc.gpsimd if nst % 2 == 0 else nc.sync
                            nst += 1
                            eng.dma_start(
                                out_d.ap()[db][:, 4 * h:4 * h + 4, :], ot[:])

    nc.compile()
    _built[mode] = nc
    return nc


def _prep_host(x, Wq1_w, Wq1_b, Wq2_w, Wq2_b, WR_w, WR_b, mode):
    f, f8 = np.float32, np.float64
    W1a = np.concatenate([Wq1_w, Wq1_b[:, None]], axis=1)   # [512, 513]
    W2a = np.concatenate([Wq2_w, Wq2_b[:, None]], axis=1)
    WRa = np.concatenate([WR_w, WR_b[:, None]], axis=1)

    khatT = (W2a.T.astype(f8) @ W1a.astype(f8)).astype(f)   # [513, 513]
    rhat = WRa.T.astype(f)                                  # [513, 512]
    khat2 = np.ascontiguousarray(
        khatT[:D, :D].reshape(4, 128, D).transpose(1, 0, 2))
    rhat2 = np.ascontiguousarray(
        rhat[:D].reshape(4, 128, D).transpose(1, 0, 2))

    sx = x.sum(axis=1, dtype=f8).astype(f)                  # [B, 512]
    sxa = np.concatenate([sx, np.full((B, 1), float(N), f)], axis=1)
    m1row = (sxa.astype(f8) @ rhat.astype(f8)).astype(f)    # [B, 512]

    # v = P_aug[512,:] = Khat_aug[512,:] @ G_aug @ Rhat, host-computable in
    # O(N*D): z = x@k[:512] + k[512];  v = [x.T z | sum z] @ Rhat
    k = (W1a[:, D].astype(f8) @ W2a.astype(f8))             # [513]
    z = x.astype(f8) @ k[:D] + k[D]                         # [B, 4096]
    u = np.concatenate([np.einsum('bn,bnd->bd', z, x.astype(f8)),
                        z.sum(axis=1)[:, None]], axis=1)    # [B, 513]
    v = (u @ rhat.astype(f8)).astype(f)                     # [B, 512]

    # scl[:, 0:4] = v, [:, 4:8] = sx, [:, 8:12] = Khat[:, 512], col-major
    scl = np.concatenate([
        v.reshape(B, 4, 128), sx.reshape(B, 4, 128),
        np.broadcast_to(khatT[D, :D].reshape(1, 4, 128), (B, 4, 128)),
    ], axis=1).transpose(0, 2, 1).astype(f)                 # [B, 128, 12]
    scl = np.ascontiguousarray(scl)
    brow = np.stack([
        np.broadcast_to(rhat[D], (B, 128, D)),
        np.repeat(m1row[:, None, :], 128, axis=1),
    ], axis=2).astype(f)                                    # [B, 128, 2, D]
    brow = np.ascontiguousarray(brow)

    dt = ml_dtypes.bfloat16 if mode == "bf16" else f
    xa2 = np.ascontiguousarray(
        x.reshape(B, NT, 128, D).transpose(0, 2, 1, 3)).astype(dt)
    xat2 = np.ascontiguousarray(
        x.transpose(0, 2, 1).reshape(B, 4, 128, NSLAB, D)).astype(dt)

    return [
        {"xa": xa2[b], "xat": xat2[b], "khat": khat2, "rhat": rhat2,
         "scl": scl[b], "brow": brow[b]}
        for b in range(B)
    ]


def _post(res):
    # out.T tiles [db, p, slab, j] -> out[slab*512+j, db*128+p]
    return np.stack([
        np.ascontiguousarray(
            res.results[b]["out"].astype(np.float32)
            .transpose(2, 3, 0, 1)).reshape(N, D)
        for b in range(B)
    ])


def kernel(x, Wq1_w, Wq1_b, Wq2_w, Wq2_b, WR_w, WR_b):
    x = np.asarray(x, dtype=np.float32)
    args = [np.asarray(a, dtype=np.float32)
            for a in (Wq1_w, Wq1_b, Wq2_w, Wq2_b, WR_w, WR_b)]
    in_maps = _prep_host(x, *args, MODE)

    nc = _build(MODE)
    # the axon-tunneled device occasionally starts in a wedged state
    # (NRT_EXEC_UNIT_UNRECOVERABLE) and recovers on the next attempt
    last_err = None
    for attempt in range(3):
        try:
            res = run_bass_kernel_spmd(nc, in_maps, core_ids=list(range(N_CORES)))
            break
        except Exception as e:  # noqa: BLE001
            last_err = e
            import time as _time
            _time.sleep(2.0)
            try:
                import jax
                jax.clear_caches()
            except Exception:
                pass
    else:
        raise last_err
    return _post(res)


# revision 25
# speedup vs baseline: 1.1209x; 1.0108x over previous
"""Trainium2 Bass kernel for GCFAgg-style block:
    q1 = x@W1.T+b1; q2 = x@W2.T+b2; r = x@WR.T+br
    out = (q1 @ q2.T) @ r        (per batch, no softmax)

Key algebraic restructuring: with x_aug = [x | 1] and W*_aug = [W* | b*],
    out = x_aug @ (Khat @ (x_aug.T @ x_aug) @ Rhat)
where Khat = W1_aug.T @ W2_aug and Rhat = WR_aug.T are tiny host-precomputed
matrices. The device computes G = x.T @ x (symmetric: upper block-triangle
on PE, lower from PE transposes), the small chain P = Khat @ G @ Rhat, and
the projection out.T = P[:512].T @ x.T + v.

Work placement:
  - v (= P_aug row 512) is host-computed in O(N*D).
  - The rank-1 augmented terms of the chain (sx (x) rhat_row512 and
    khat_col512 (x) m1row) are materialized by the Scalar engine during the
    G window (scale-by-per-partition-scalar of a host-broadcast row) and
    folded into the chain's PSUM->SBUF copies as DVE adds — no K=1 PE
    matmuls.
  - G-symmetry: M1 groups run in order g1=3..0; group 3 needs only upper
    blocks, and each PE transpose that fills a lower block is interleaved
    right before the first group that consumes it.
  - out.T orientation makes +v a per-partition bias fused into the Scalar
    engine PSUM->SBUF copy; host reassembles the transposed output.

Perf notes (per core, PE @2.4GHz, ~332GB/s HBM):
  - PE ~127k cycles = 53us is the floor (G 41k, chain ~18k, out 65.5k).
  - bf16 x streams and P buy DMA bytes (bf16/f32r matmul are both
    1 cycle/row); the chain stays f32r for accuracy.
  - DMA triggers cost ~600ns of queue-engine time each regardless of size,
    so tiles move in large batched triggers; xa prefetch depth (pool bufs)
    covers all 10 triggers so the stream never backpressures.
  - A few warmup matmuls run during the initial DMA wait to pre-ramp the
    PE clock (0.65/1.2 GHz p-states before 3us of continuous work).

Sharding: batch dim B=8, one batch per NeuronCore (data parallel).

Self-contained: hardcodes shapes from the problem spec
(x: [8, 4096, 512] f32; W*: [512, 512]; b*: [512]).
"""
import os
import sys

sys.path.insert(0, "/opt/trn_rl_repo")

import numpy as np
import ml_dtypes

import concourse.bass as bass
import concourse.mybir as mybir
import concourse.tile as tile
from concourse import bacc
from concourse.bass_utils import run_bass_kernel_spmd
from concourse.masks import make_identity
from concourse.tile_rust import add_dep_helper

B = 8          # batch -> one per core
N = 4096       # tokens per batch
D = 512        # model dim
NT = N // 128  # 32 row tiles
NSLAB = 8      # 512-token slabs for the out.T phase
N_CORES = 8

F32 = mybir.dt.float32
F32R = mybir.dt.float32r
BF16 = mybir.dt.bfloat16

# mode: "bf16" (bf16 x/P storage+matmul, f32r chain) or "f32r"
MODE = os.environ.get("GCF_MODE", "bf16")

# xa trigger batching: first single tile goes on gpsimd (earliest-ready
# queue), the rest stream on sync
XA_BATCHES = [1, 1, 2, 4, 4, 4, 4, 4, 4, 4]
N_WARM = 10

_built = {}


def _build(mode):
    if mode in _built:
        return _built[mode]

    big = BF16 if mode == "bf16" else F32R
    chain = F32R

    def mm_ap(ap, dt):
        return ap if ap.dtype == dt else ap.bitcast(dt)

    nc = bacc.Bacc("TRN2", target_bir_lowering=False, debug=False,
                   num_devices=N_CORES)

    # all DRAM tensors are laid out partition-dim first by the host
    xa_d = nc.dram_tensor("xa", (128, NT, D), big, kind="ExternalInput")
    xat_d = nc.dram_tensor("xat", (4, 128, NSLAB, D), big, kind="ExternalInput")
    khat_d = nc.dram_tensor("khat", (128, 4, D), chain, kind="ExternalInput")
    rhat_d = nc.dram_tensor("rhat", (128, 4, D), chain, kind="ExternalInput")
    # scl[:, 0:4] = v (bias), [:, 4:8] = sx columns, [:, 8:12] = khat col 512
    scl_d = nc.dram_tensor("scl", (128, 12), F32, kind="ExternalInput")
    # broadcast rows: [0] = Rhat row 512, [1] = m1row (both repl. 128x)
    brow_d = nc.dram_tensor("brow", (128, 2, D), F32, kind="ExternalInput")
    out_d = nc.dram_tensor("out", (4, 128, NSLAB, D), BF16,
                           kind="ExternalOutput")

    with tile.TileContext(nc) as tc:
        with (
            tc.tile_pool(name="xa", bufs=len(XA_BATCHES)) as xa_pool,
            tc.tile_pool(name="const", bufs=1) as const_pool,
            tc.tile_pool(name="gsb", bufs=1) as g_pool,
            tc.tile_pool(name="chain", bufs=1) as chain_pool,
            tc.tile_pool(name="outsb", bufs=3) as out_pool,
        ):
            # ---- xa stream on sync (first trigger fires earliest there) ----
            xa_tiles = []          # (tile, sub-index) per global row tile
            t0 = 0
            for bi, nb in enumerate(XA_BATCHES):
                xa_t = xa_pool.tile([128, 4, D], big, tag="xa")
                nc.sync.dma_start(xa_t[:, :nb, :], xa_d.ap()[:, t0:t0 + nb, :])
                for j in range(nb):
                    xa_tiles.append((xa_t, j))
                t0 += nb

            ident = const_pool.tile([128, 128], F32, tag="ident")
            make_identity(nc, ident[:])
            warm_sb = const_pool.tile([128, D], big, tag="warm")
            nc.vector.memset(warm_sb[:], 0.0)

            khat_sb = const_pool.tile([128, 4, D], chain, tag="khat")
            rhat_sb = const_pool.tile([128, 4, D], chain, tag="rhat")
            scl_sb = const_pool.tile([128, 12], F32, tag="scl")
            brow_sb = const_pool.tile([128, 2, D], F32, tag="brow")

            # ---- phase 1: G = x^T @ x; upper block-triangle only ----
            g_sb = [g_pool.tile([128, D], chain, tag=f"g{c}", name=f"g{c}")
                    for c in range(4)]
            with tc.tile_pool(name="psG", bufs=1, space="PSUM") as psG_pool:
                # warmup matmuls: pre-ramp the PE clock while the first xa
                # tiles are still in flight (results unused)
                ps_w = psG_pool.tile([128, D], F32, tag="warm")
                for _ in range(N_WARM):
                    nc.tensor.matmul(ps_w[:], mm_ap(warm_sb[:, :128], big),
                                     mm_ap(warm_sb[:], big),
                                     start=True, stop=True)

                ps_ga = [psG_pool.tile([128, D - c * 128], F32, tag=f"ga{c}",
                                       name=f"ga{c}") for c in range(4)]
                gate_mms = []
                for t in range(NT):
                    xa_t, j = xa_tiles[t]
                    for c in range(4):
                        mm = nc.tensor.matmul(
                            ps_ga[c][:],
                            mm_ap(xa_t[:, j, c * 128:(c + 1) * 128], big),
                            mm_ap(xa_t[:, j, c * 128:D], big),
                            start=(t == 0), stop=(t == NT - 1),
                        )
                        if c == 3:
                            gate_mms.append(mm)

                # consts on gpsimd, gated by first use so they never crowd
                # the xa stream: rhat feeds the first M1 group right at G
                # end; khat is first needed ~6us later at the P groups;
                # scl/brow at the chain adds
                for cd in (nc.gpsimd.dma_start(scl_sb[:], scl_d.ap()[:]),
                           nc.gpsimd.dma_start(brow_sb[:], brow_d.ap()[:])):
                    add_dep_helper(cd.ins, gate_mms[8].ins,
                                   reason="scl/brow (tiny, needed first) t=8")
                cd = nc.gpsimd.dma_start(rhat_sb[:], rhat_d.ap()[:])
                add_dep_helper(cd.ins, gate_mms[10].ins,
                               reason="rhat gated behind G t=10")
                cd = nc.gpsimd.dma_start(khat_sb[:], khat_d.ap()[:])
                add_dep_helper(cd.ins, gate_mms[20].ins,
                               reason="khat gated behind G t=20")

                # xat loads (sync queue, after the xa triggers in program
                # order) gated behind the G tail: during G the xa stream +
                # consts saturate HBM; the chain window is otherwise idle.
                xat_sb = [const_pool.tile([128, NSLAB, D], big, tag=f"xat{c}",
                                          name=f"xat{c}") for c in range(4)]
                for h in range(2):
                    for c in range(4):
                        xd = nc.sync.dma_start(
                            xat_sb[c][:, 4 * h:4 * h + 4, :],
                            xat_d.ap()[c][:, 4 * h:4 * h + 4, :])
                        add_dep_helper(xd.ins, gate_mms[26 if h == 0 else 31].ins,
                                       reason="xat gated behind G tail")

                # ---- phase 2 interleaved with G wrap-up; the upper-block
                # PSUM->SBUF copies are emitted just-in-time per column so
                # the DVE backlog never stalls the next M1 group ----
                # the copies run on the Scalar engine (idle until phase 3,
                # and it CAN read PSUM) so the DVE queue holds only the
                # chain adds — neither engine's backlog stalls the PE
                def copy_col(g1):
                    for c in range(g1 + 1):
                        nc.scalar.copy(
                            g_sb[c][:, g1 * 128:(g1 + 1) * 128],
                            ps_ga[c][:, (g1 - c) * 128:(g1 - c + 1) * 128])

                with tc.tile_pool(name="psC", bufs=2, space="PSUM") as psC_pool:
                    m1_sb = [chain_pool.tile([128, D], chain, tag=f"m1{c}",
                                             name=f"m1{c}") for c in range(4)]

                    def transpose_block(c1, c2):
                        # fill lower block (c2, c1) from upper (c1, c2)
                        ps_tr = psC_pool.tile([128, 128], F32, tag="tr", bufs=1)
                        nc.tensor.transpose(
                            ps_tr[:],
                            mm_ap(g_sb[c1][:, c2 * 128:(c2 + 1) * 128], F32),
                            ident[:],
                        )
                        # tr copies go on DVE (idle until the chain adds) so
                        # they are not queued behind scalar's column copies
                        nc.vector.tensor_copy(
                            g_sb[c2][:, c1 * 128:(c1 + 1) * 128], ps_tr[:])

                    # M1 groups g1 = 3..0; PE transposes that fill the lower
                    # blocks needed by group g1 are emitted just before it
                    for g1 in range(3, -1, -1):
                        copy_col(g1)
                        if g1 == 2:
                            transpose_block(2, 3)
                        elif g1 == 1:
                            transpose_block(1, 2)
                            transpose_block(1, 3)
                        elif g1 == 0:
                            transpose_block(0, 1)
                            transpose_block(0, 2)
                            transpose_block(0, 3)
                        ps = psC_pool.tile([128, D], F32, tag="chain", bufs=2)
                        for i, g2 in enumerate(
                                list(range(g1 + 1)) + list(range(g1 + 1, 4))):
                            nc.tensor.matmul(
                                ps[:],
                                mm_ap(g_sb[g2][:, g1 * 128:(g1 + 1) * 128],
                                      chain),
                                mm_ap(rhat_sb[:, g2, :], chain),
                                start=(i == 0), stop=(i == 3),
                            )
                        # m1 = ps + sx[g1-block] (x) Rhat[512,:] — the rank-1
                        # augmented term folds into the PSUM->SBUF copy
                        nc.vector.scalar_tensor_tensor(
                            m1_sb[g1][:], brow_sb[:, 0, :],
                            scl_sb[:, 4 + g1:5 + g1], ps[:],
                            mybir.AluOpType.mult, mybir.AluOpType.add)

                    p_sb = [chain_pool.tile([128, D], big, tag=f"p{c}",
                                            name=f"p{c}") for c in range(4)]
                    for g1 in range(4):
                        ps = psC_pool.tile([128, D], F32, tag="chain", bufs=2)
                        # g2 order 3..0: m1 adds complete in that order, so
                        # the group never waits on the most recent add
                        for i, g2 in enumerate(range(3, -1, -1)):
                            nc.tensor.matmul(
                                ps[:],
                                mm_ap(khat_sb[:, g2, g1 * 128:(g1 + 1) * 128],
                                      chain),
                                mm_ap(m1_sb[g2][:], chain),
                                start=(i == 0), stop=(i == 3),
                            )
                        # p = ps + Khat[g1-block, 512] (x) m1row
                        nc.vector.scalar_tensor_tensor(
                            p_sb[g1][:], brow_sb[:, 1, :],
                            scl_sb[:, 8 + g1:9 + g1], ps[:],
                            mybir.AluOpType.mult, mybir.AluOpType.add)

            # ---- phase 3: out.T[db,:] = sum_c P[c,db].T @ x.T[c,:] + v[db]
            # (+v fused into the Scalar-engine PSUM->SBUF copy as a
            # per-partition bias) ----
            with tc.tile_pool(name="psO", bufs=1, space="PSUM") as psO_pool:
                nst = 0
                for h in range(2):
                    for db in range(4):
                        ot = out_pool.tile([128, 4, D], BF16, tag="ot")
                        for si in range(4):
                            s = 4 * h + si
                            ps = psO_pool.tile([128, D], F32, tag="out", bufs=6)
                            for c in range(4):
                                nc.tensor.matmul(
                                    ps[:],
                                    mm_ap(p_sb[c][:, db * 128:(db + 1) * 128],
                                          big),
                                    mm_ap(xat_sb[c][:, s, :], big),
                                    start=(c == 0), stop=(c == 3),
                                )
                            nc.scalar.add(ot[:, si, :], ps[:],
                                          scl_sb[:, db:db + 1])
                            # split the final buffer's store (2+1+1 slabs) so
                            # the very last transfer is short
                            if h == 1 and db == 3 and si >= 1:
                                eng = nc.gpsimd if nst % 2 == 0 else nc.sync
                                nst += 1
                                lo, n = (0, 2) if si == 1 else (si, 1)
                                eng.dma_start(
                                    out_d.ap()[db][:, 4 * h + lo:4 * h + lo + n, :],
                                    ot[:, lo:lo + n, :])
                        if not (h == 1 and db == 3):
                            eng = nc.gpsimd if nst % 2 == 0 else nc.sync
                            nst += 1
                            eng.dma_start(
                                out_d.ap()[db][:, 4 * h:4 * h + 4, :], ot[:])

    nc.compile()
    _built[mode] = nc
    return nc


def _prep_host(x, Wq1_w, Wq1_b, Wq2_w, Wq2_b, WR_w, WR_b, mode):
    f, f8 = np.float32, np.float64
    W1a = np.concatenate([Wq1_w, Wq1_b[:, None]], axis=1)   # [512, 513]
    W2a = np.concatenate([Wq2_w, Wq2_b[:, None]], axis=1)
    WRa = np.concatenate([WR_w, WR_b[:, None]], axis=1)

    khatT = (W2a.T.astype(f8) @ W1a.astype(f8)).astype(f)   # [513, 513]
    rhat = WRa.T.astype(f)                                  # [513, 512]
    khat2 = np.ascontiguousarray(
        khatT[:D, :D].reshape(4, 128, D).transpose(1, 0, 2))
    rhat2 = np.ascontiguousarray(
        rhat[:D].reshape(4, 128, D).transpose(1, 0, 2))

    sx = x.sum(axis=1, dtype=f8).astype(f)                  # [B, 512]
    sxa = np.concatenate([sx, np.full((B, 1), float(N), f)], axis=1)
    m1row = (sxa.astype(f8) @ rhat.astype(f8)).astype(f)    # [B, 512]

    # v = P_aug[512,:] = Khat_aug[512,:] @ G_aug @ Rhat, host-computable in
    # O(N*D): z = x@k[:512] + k[512];  v = [x.T z | sum z] @ Rhat
    k = (W1a[:, D].astype(f8) @ W2a.astype(f8))             # [513]
    z = x.astype(f8) @ k[:D] + k[D]                         # [B, 4096]
    u = np.concatenate([np.einsum('bn,bnd->bd', z, x.astype(f8)),
                        z.sum(axis=1)[:, None]], axis=1)    # [B, 513]
    v = (u @ rhat.astype(f8)).astype(f)                     # [B, 512]

    # scl[:, 0:4] = v, [:, 4:8] = sx, [:, 8:12] = Khat[:, 512], col-major
    scl = np.concatenate([
        v.reshape(B, 4, 128), sx.reshape(B, 4, 128),
        np.broadcast_to(khatT[D, :D].reshape(1, 4, 128), (B, 4, 128)),
    ], axis=1).transpose(0, 2, 1).astype(f)                 # [B, 128, 12]
    scl = np.ascontiguousarray(scl)
    brow = np.stack([
        np.broadcast_to(rhat[D], (B, 128, D)),
        np.repeat(m1row[:, None, :], 128, axis=1),
    ], axis=2).astype(f)                                    # [B, 128, 2, D]
    brow = np.ascontiguousarray(brow)

    dt = ml_dtypes.bfloat16 if mode == "bf16" else f
    xa2 = np.ascontiguousarray(
        x.reshape(B, NT, 128, D).transpose(0, 2, 1, 3)).astype(dt)
    xat2 = np.ascontiguousarray(
        x.transpose(0, 2, 1).reshape(B, 4, 128, NSLAB, D)).astype(dt)

    return [
        {"xa": xa2[b], "xat": xat2[b], "khat": khat2, "rhat": rhat2,
         "scl": scl[b], "brow": brow[b]}
        for b in range(B)
    ]


def _post(res):
    # out.T tiles [db, p, slab, j] -> out[slab*512+j, db*128+p]
    return np.stack([
        np.ascontiguousarray(
            res.results[b]["out"].astype(np.float32)
            .transpose(2, 3, 0, 1)).reshape(N, D)
        for b in range(B)
    ])


def kernel(x, Wq1_w, Wq1_b, Wq2_w, Wq2_b, WR_w, WR_b):
    x = np.asarray(x, dtype=np.float32)
    args = [np.asarray(a, dtype=np.float32)
            for a in (Wq1_w, Wq1_b, Wq2_w, Wq2_b, WR_w, WR_b)]
    in_maps = _prep_host(x, *args, MODE)

    nc = _build(MODE)
    # the axon-tunneled device occasionally starts in a wedged state
    # (NRT_EXEC_UNIT_UNRECOVERABLE) and recovers on the next attempt
    last_err = None
    for attempt in range(3):
        try:
            res = run_bass_kernel_spmd(nc, in_maps, core_ids=list(range(N_CORES)))
            break
        except Exception as e:  # noqa: BLE001
            last_err = e
            import time as _time
            _time.sleep(2.0)
            try:
                import jax
                jax.clear_caches()
            except Exception:
                pass
    else:
        raise last_err
    return _post(res)


# revision 28
# speedup vs baseline: 1.1284x; 1.0067x over previous
"""Trainium2 Bass kernel for GCFAgg-style block:
    q1 = x@W1.T+b1; q2 = x@W2.T+b2; r = x@WR.T+br
    out = (q1 @ q2.T) @ r        (per batch, no softmax)

Key algebraic restructuring: with x_aug = [x | 1] and W*_aug = [W* | b*],
    out = x_aug @ (Khat @ (x_aug.T @ x_aug) @ Rhat)
where Khat = W1_aug.T @ W2_aug and Rhat = WR_aug.T are tiny host-precomputed
matrices. The device computes G = x.T @ x (symmetric: upper block-triangle
on PE, lower from PE transposes), the small chain P = Khat @ G @ Rhat, and
the projection out.T = P[:512].T @ x.T + v.

Work placement:
  - v (= P_aug row 512) is host-computed in O(N*D).
  - The rank-1 augmented terms of the chain (sx (x) rhat_row512 and
    khat_col512 (x) m1row) are materialized by the Scalar engine during the
    G window (scale-by-per-partition-scalar of a host-broadcast row) and
    folded into the chain's PSUM->SBUF copies as DVE adds — no K=1 PE
    matmuls.
  - G-symmetry: M1 groups run in order g1=3..0; group 3 needs only upper
    blocks, and each PE transpose that fills a lower block is interleaved
    right before the first group that consumes it.
  - out.T orientation makes +v a per-partition bias fused into the Scalar
    engine PSUM->SBUF copy; host reassembles the transposed output.

Perf notes (per core, PE @2.4GHz, ~332GB/s HBM):
  - PE ~127k cycles = 53us is the floor (G 41k, chain ~18k, out 65.5k).
  - bf16 x streams and P buy DMA bytes (bf16/f32r matmul are both
    1 cycle/row); the chain stays f32r for accuracy.
  - DMA triggers cost ~600ns of queue-engine time each regardless of size,
    so tiles move in large batched triggers; xa prefetch depth (pool bufs)
    covers all 10 triggers so the stream never backpressures.
  - A few warmup matmuls run during the initial DMA wait to pre-ramp the
    PE clock (0.65/1.2 GHz p-states before 3us of continuous work).

Sharding: batch dim B=8, one batch per NeuronCore (data parallel).

Self-contained: hardcodes shapes from the problem spec
(x: [8, 4096, 512] f32; W*: [512, 512]; b*: [512]).
"""
import os
import sys

sys.path.insert(0, "/opt/trn_rl_repo")

import numpy as np
import ml_dtypes

import concourse.bass as bass
import concourse.mybir as mybir
import concourse.tile as tile
from concourse import bacc
from concourse.bass_utils import run_bass_kernel_spmd
from concourse.masks import make_identity
from concourse.tile_rust import add_dep_helper

B = 8          # batch -> one per core
N = 4096       # tokens per batch
D = 512        # model dim
NT = N // 128  # 32 row tiles
NSLAB = 8      # 512-token slabs for the out.T phase
N_CORES = 8

F32 = mybir.dt.float32
F32R = mybir.dt.float32r
BF16 = mybir.dt.bfloat16

# mode: "bf16" (bf16 x/P storage+matmul, f32r chain) or "f32r"
MODE = os.environ.get("GCF_MODE", "bf16")

# xa trigger batching: first single tile goes on gpsimd (earliest-ready
# queue), the rest stream on sync
XA_BATCHES = [1, 1, 2, 4, 4, 4, 4, 4, 4, 4]
N_WARM = 10

_built = {}


def _build(mode):
    if mode in _built:
        return _built[mode]

    big = BF16 if mode == "bf16" else F32R
    chain = F32R

    def mm_ap(ap, dt):
        return ap if ap.dtype == dt else ap.bitcast(dt)

    nc = bacc.Bacc("TRN2", target_bir_lowering=False, debug=False,
                   num_devices=N_CORES)

    # all DRAM tensors are laid out partition-dim first by the host
    xa_d = nc.dram_tensor("xa", (128, NT, D), big, kind="ExternalInput")
    xat_d = nc.dram_tensor("xat", (4, 128, NSLAB, D), big, kind="ExternalInput")
    khat_d = nc.dram_tensor("khat", (128, 4, D), chain, kind="ExternalInput")
    rhat_d = nc.dram_tensor("rhat", (128, 4, D), chain, kind="ExternalInput")
    # scl[:, 0:4] = v (bias), [:, 4:8] = sx columns, [:, 8:12] = khat col 512
    scl_d = nc.dram_tensor("scl", (128, 12), F32, kind="ExternalInput")
    # broadcast rows: [0] = Rhat row 512, [1] = m1row (both repl. 128x)
    brow_d = nc.dram_tensor("brow", (128, 2, D), F32, kind="ExternalInput")
    out_d = nc.dram_tensor("out", (4, 128, NSLAB, D), BF16,
                           kind="ExternalOutput")

    with tile.TileContext(nc) as tc:
        with (
            tc.tile_pool(name="xa", bufs=len(XA_BATCHES)) as xa_pool,
            tc.tile_pool(name="const", bufs=1) as const_pool,
            tc.tile_pool(name="gsb", bufs=1) as g_pool,
            tc.tile_pool(name="chain", bufs=1) as chain_pool,
            tc.tile_pool(name="outsb", bufs=3) as out_pool,
        ):
            # ---- xa stream on sync (first trigger fires earliest there) ----
            xa_tiles = []          # (tile, sub-index) per global row tile
            t0 = 0
            for bi, nb in enumerate(XA_BATCHES):
                xa_t = xa_pool.tile([128, 4, D], big, tag="xa")
                nc.sync.dma_start(xa_t[:, :nb, :], xa_d.ap()[:, t0:t0 + nb, :])
                for j in range(nb):
                    xa_tiles.append((xa_t, j))
                t0 += nb

            ident = const_pool.tile([128, 128], F32, tag="ident")
            make_identity(nc, ident[:])
            warm_sb = const_pool.tile([128, D], big, tag="warm")
            nc.vector.memset(warm_sb[:], 0.0)

            khat_sb = const_pool.tile([128, 4, D], chain, tag="khat")
            rhat_sb = const_pool.tile([128, 4, D], chain, tag="rhat")
            scl_sb = const_pool.tile([128, 12], F32, tag="scl")
            brow_sb = const_pool.tile([128, 2, D], F32, tag="brow")

            # ---- phase 1: G = x^T @ x; upper block-triangle only ----
            g_sb = [g_pool.tile([128, D], chain, tag=f"g{c}", name=f"g{c}")
                    for c in range(4)]
            with tc.tile_pool(name="psG", bufs=1, space="PSUM") as psG_pool:
                ps_ga = [psG_pool.tile([128, D - c * 128], F32, tag=f"ga{c}",
                                       name=f"ga{c}") for c in range(4)]
                # warmup matmuls: pre-ramp the PE clock while the first xa
                # tiles are still in flight (results unused; they borrow
                # ga0's bank, which G's first start=True resets anyway)
                for _ in range(N_WARM):
                    nc.tensor.matmul(ps_ga[0][:], mm_ap(warm_sb[:, :128], big),
                                     mm_ap(warm_sb[:], big),
                                     start=True, stop=True,
                                     skip_group_check=True)
                gate_mms = []
                for t in range(NT):
                    xa_t, j = xa_tiles[t]
                    for c in range(4):
                        mm = nc.tensor.matmul(
                            ps_ga[c][:],
                            mm_ap(xa_t[:, j, c * 128:(c + 1) * 128], big),
                            mm_ap(xa_t[:, j, c * 128:D], big),
                            start=(t == 0), stop=(t == NT - 1),
                        )
                        if c == 3:
                            gate_mms.append(mm)

                # consts on gpsimd, gated by first use so they never crowd
                # the xa stream: rhat feeds the first M1 group right at G
                # end; khat is first needed ~6us later at the P groups;
                # scl/brow at the chain adds
                for cd in (nc.gpsimd.dma_start(scl_sb[:], scl_d.ap()[:]),
                           nc.gpsimd.dma_start(brow_sb[:], brow_d.ap()[:])):
                    add_dep_helper(cd.ins, gate_mms[8].ins,
                                   reason="scl/brow (tiny, needed first) t=8")
                cd = nc.gpsimd.dma_start(rhat_sb[:], rhat_d.ap()[:])
                add_dep_helper(cd.ins, gate_mms[10].ins,
                               reason="rhat gated behind G t=10")
                cd = nc.gpsimd.dma_start(khat_sb[:], khat_d.ap()[:])
                add_dep_helper(cd.ins, gate_mms[20].ins,
                               reason="khat gated behind G t=20")

                # xat loads (sync queue, after the xa triggers in program
                # order) gated behind the G tail: during G the xa stream +
                # consts saturate HBM; the chain window is otherwise idle.
                xat_sb = [const_pool.tile([128, NSLAB, D], big, tag=f"xat{c}",
                                          name=f"xat{c}") for c in range(4)]
                for h in range(2):
                    for c in range(4):
                        xd = nc.sync.dma_start(
                            xat_sb[c][:, 4 * h:4 * h + 4, :],
                            xat_d.ap()[c][:, 4 * h:4 * h + 4, :])
                        add_dep_helper(xd.ins, gate_mms[26 if h == 0 else 31].ins,
                                       reason="xat gated behind G tail")

                # ---- phase 2 interleaved with G wrap-up; the upper-block
                # PSUM->SBUF copies are emitted just-in-time per column so
                # the DVE backlog never stalls the next M1 group ----
                # the copies run on the Scalar engine (idle until phase 3,
                # and it CAN read PSUM) so the DVE queue holds only the
                # chain adds — neither engine's backlog stalls the PE
                def copy_col(g1):
                    for c in range(g1 + 1):
                        nc.scalar.copy(
                            g_sb[c][:, g1 * 128:(g1 + 1) * 128],
                            ps_ga[c][:, (g1 - c) * 128:(g1 - c + 1) * 128])

                with tc.tile_pool(name="psC", bufs=2, space="PSUM") as psC_pool:
                    m1_sb = [chain_pool.tile([128, D], chain, tag=f"m1{c}",
                                             name=f"m1{c}") for c in range(4)]

                    def transpose_block(c1, c2):
                        # fill lower block (c2, c1) from upper (c1, c2)
                        ps_tr = psC_pool.tile([128, 128], F32, tag="tr", bufs=1)
                        nc.tensor.transpose(
                            ps_tr[:],
                            mm_ap(g_sb[c1][:, c2 * 128:(c2 + 1) * 128], F32),
                            ident[:],
                        )
                        # tr copies go on DVE (idle until the chain adds) so
                        # they are not queued behind scalar's column copies
                        nc.vector.tensor_copy(
                            g_sb[c2][:, c1 * 128:(c1 + 1) * 128], ps_tr[:])

                    # M1 groups g1 = 3..0; PE transposes run one group ahead
                    # of first use so their DVE copies are never on the
                    # critical path
                    for g1 in range(3, -1, -1):
                        copy_col(g1)
                        if g1 == 2:
                            transpose_block(2, 3)
                            transpose_block(1, 2)
                            transpose_block(1, 3)
                        elif g1 == 1:
                            transpose_block(0, 1)
                            transpose_block(0, 2)
                            transpose_block(0, 3)
                        ps = psC_pool.tile([128, D], F32, tag="chain", bufs=3)
                        for i, g2 in enumerate(
                                list(range(g1 + 1)) + list(range(g1 + 1, 4))):
                            nc.tensor.matmul(
                                ps[:],
                                mm_ap(g_sb[g2][:, g1 * 128:(g1 + 1) * 128],
                                      chain),
                                mm_ap(rhat_sb[:, g2, :], chain),
                                start=(i == 0), stop=(i == 3),
                            )
                        # m1 = ps + sx[g1-block] (x) Rhat[512,:] — the rank-1
                        # augmented term folds into the PSUM->SBUF copy
                        nc.vector.scalar_tensor_tensor(
                            m1_sb[g1][:], brow_sb[:, 0, :],
                            scl_sb[:, 4 + g1:5 + g1], ps[:],
                            mybir.AluOpType.mult, mybir.AluOpType.add)

                    p_sb = [chain_pool.tile([128, D], big, tag=f"p{c}",
                                            name=f"p{c}") for c in range(4)]
                    for g1 in range(4):
                        ps = psC_pool.tile([128, D], F32, tag="chain", bufs=3)
                        # g2 order 3..0: m1 adds complete in that order, so
                        # the group never waits on the most recent add
                        for i, g2 in enumerate(range(3, -1, -1)):
                            nc.tensor.matmul(
                                ps[:],
                                mm_ap(khat_sb[:, g2, g1 * 128:(g1 + 1) * 128],
                                      chain),
                                mm_ap(m1_sb[g2][:], chain),
                                start=(i == 0), stop=(i == 3),
                            )
                        # p = ps + Khat[g1-block, 512] (x) m1row
                        nc.vector.scalar_tensor_tensor(
                            p_sb[g1][:], brow_sb[:, 1, :],
                            scl_sb[:, 8 + g1:9 + g1], ps[:],
                            mybir.AluOpType.mult, mybir.AluOpType.add)

            # ---- phase 3: out.T[db,:] = sum_c P[c,db].T @ x.T[c,:] + v[db]
            # (+v fused into the Scalar-engine PSUM->SBUF copy as a
            # per-partition bias) ----
            with tc.tile_pool(name="psO", bufs=1, space="PSUM") as psO_pool:
                nst = 0
                for h in range(2):
                    for db in range(4):
                        ot = out_pool.tile([128, 4, D], BF16, tag="ot")
                        for si in range(4):
                            s = 4 * h + si
                            ps = psO_pool.tile([128, D], F32, tag="out", bufs=6)
                            for c in range(4):
                                nc.tensor.matmul(
                                    ps[:],
                                    mm_ap(p_sb[c][:, db * 128:(db + 1) * 128],
                                          big),
                                    mm_ap(xat_sb[c][:, s, :], big),
                                    start=(c == 0), stop=(c == 3),
                                )
                            nc.scalar.add(ot[:, si, :], ps[:],
                                          scl_sb[:, db:db + 1])
                            # split the final buffer's store (2+1+1 slabs) so
                            # the very last transfer is short
                            if h == 1 and db == 3 and si >= 1:
                                eng = nc.gpsimd if nst % 2 == 0 else nc.sync
                                nst += 1
                                lo, n = (0, 2) if si == 1 else (si, 1)
                                eng.dma_start(
                                    out_d.ap()[db][:, 4 * h + lo:4 * h + lo + n, :],
                                    ot[:, lo:lo + n, :])
                        if not (h == 1 and db == 3):
                            eng = nc.gpsimd if nst % 2 == 0 else nc.sync
                            nst += 1
                            eng.dma_start(
                                out_d.ap()[db][:, 4 * h:4 * h + 4, :], ot[:])

    nc.compile()
    _built[mode] = nc
    return nc


def _prep_host(x, Wq1_w, Wq1_b, Wq2_w, Wq2_b, WR_w, WR_b, mode):
    f, f8 = np.float32, np.float64
    W1a = np.concatenate([Wq1_w, Wq1_b[:, None]], axis=1)   # [512, 513]
    W2a = np.concatenate([Wq2_w, Wq2_b[:, None]], axis=1)
    WRa = np.concatenate([WR_w, WR_b[:, None]], axis=1)

    khatT = (W2a.T.astype(f8) @ W1a.astype(f8)).astype(f)   # [513, 513]
    rhat = WRa.T.astype(f)                                  # [513, 512]
    khat2 = np.ascontiguousarray(
        khatT[:D, :D].reshape(4, 128, D).transpose(1, 0, 2))
    rhat2 = np.ascontiguousarray(
        rhat[:D].reshape(4, 128, D).transpose(1, 0, 2))

    sx = x.sum(axis=1, dtype=f8).astype(f)                  # [B, 512]
    sxa = np.concatenate([sx, np.full((B, 1), float(N), f)], axis=1)
    m1row = (sxa.astype(f8) @ rhat.astype(f8)).astype(f)    # [B, 512]

    # v = P_aug[512,:] = Khat_aug[512,:] @ G_aug @ Rhat, host-computable in
    # O(N*D): z = x@k[:512] + k[512];  v = [x.T z | sum z] @ Rhat
    k = (W1a[:, D].astype(f8) @ W2a.astype(f8))             # [513]
    z = x.astype(f8) @ k[:D] + k[D]                         # [B, 4096]
    u = np.concatenate([np.einsum('bn,bnd->bd', z, x.astype(f8)),
                        z.sum(axis=1)[:, None]], axis=1)    # [B, 513]
    v = (u @ rhat.astype(f8)).astype(f)                     # [B, 512]

    # scl[:, 0:4] = v, [:, 4:8] = sx, [:, 8:12] = Khat[:, 512], col-major
    scl = np.concatenate([
        v.reshape(B, 4, 128), sx.reshape(B, 4, 128),
        np.broadcast_to(khatT[D, :D].reshape(1, 4, 128), (B, 4, 128)),
    ], axis=1).transpose(0, 2, 1).astype(f)                 # [B, 128, 12]
    scl = np.ascontiguousarray(scl)
    brow = np.stack([
        np.broadcast_to(rhat[D], (B, 128, D)),
        np.repeat(m1row[:, None, :], 128, axis=1),
    ], axis=2).astype(f)                                    # [B, 128, 2, D]
    brow = np.ascontiguousarray(brow)

    dt = ml_dtypes.bfloat16 if mode == "bf16" else f
    xa2 = np.ascontiguousarray(
        x.reshape(B, NT, 128, D).transpose(0, 2, 1, 3)).astype(dt)
    xat2 = np.ascontiguousarray(
        x.transpose(0, 2, 1).reshape(B, 4, 128, NSLAB, D)).astype(dt)

    return [
        {"xa": xa2[b], "xat": xat2[b], "khat": khat2, "rhat": rhat2,
         "scl": scl[b], "brow": brow[b]}
        for b in range(B)
    ]


def _post(res):
    # out.T tiles [db, p, slab, j] -> out[slab*512+j, db*128+p]
    return np.stack([
        np.ascontiguousarray(
            res.results[b]["out"].astype(np.float32)
            .transpose(2, 3, 0, 1)).reshape(N, D)
        for b in range(B)
    ])


def kernel(x, Wq1_w, Wq1_b, Wq2_w, Wq2_b, WR_w, WR_b):
    x = np.asarray(x, dtype=np.float32)
    args = [np.asarray(a, dtype=np.float32)
            for a in (Wq1_w, Wq1_b, Wq2_w, Wq2_b, WR_w, WR_b)]
    in_maps = _prep_host(x, *args, MODE)

    nc = _build(MODE)
    # the axon-tunneled device occasionally starts in a wedged state
    # (NRT_EXEC_UNIT_UNRECOVERABLE) and recovers on the next attempt
    last_err = None
    for attempt in range(3):
        try:
            res = run_bass_kernel_spmd(nc, in_maps, core_ids=list(range(N_CORES)))
            break
        except Exception as e:  # noqa: BLE001
            last_err = e
            import time as _time
            _time.sleep(2.0)
            try:
                import jax
                jax.clear_caches()
            except Exception:
                pass
    else:
        raise last_err
    return _post(res)


# revision 29
# speedup vs baseline: 1.1424x; 1.0124x over previous
"""Trainium2 Bass kernel for GCFAgg-style block:
    q1 = x@W1.T+b1; q2 = x@W2.T+b2; r = x@WR.T+br
    out = (q1 @ q2.T) @ r        (per batch, no softmax)

Key algebraic restructuring: with x_aug = [x | 1] and W*_aug = [W* | b*],
    out = x_aug @ (Khat @ (x_aug.T @ x_aug) @ Rhat)
where Khat = W1_aug.T @ W2_aug and Rhat = WR_aug.T are tiny host-precomputed
matrices. The device computes G = x.T @ x (symmetric: upper block-triangle
on PE, lower from PE transposes), the small chain P = Khat @ G @ Rhat, and
the projection out.T = P[:512].T @ x.T + v.

Work placement:
  - v (= P_aug row 512) is host-computed in O(N*D).
  - The rank-1 augmented terms of the chain (sx (x) rhat_row512 and
    khat_col512 (x) m1row) are materialized by the Scalar engine during the
    G window (scale-by-per-partition-scalar of a host-broadcast row) and
    folded into the chain's PSUM->SBUF copies as DVE adds — no K=1 PE
    matmuls.
  - G-symmetry: M1 groups run in order g1=3..0; group 3 needs only upper
    blocks, and each PE transpose that fills a lower block is interleaved
    right before the first group that consumes it.
  - out.T orientation makes +v a per-partition bias fused into the Scalar
    engine PSUM->SBUF copy; host reassembles the transposed output.

Perf notes (per core, PE @2.4GHz, ~332GB/s HBM):
  - PE ~127k cycles = 53us is the floor (G 41k, chain ~18k, out 65.5k).
  - bf16 x streams and P buy DMA bytes (bf16/f32r matmul are both
    1 cycle/row); the chain stays f32r for accuracy.
  - DMA triggers cost ~600ns of queue-engine time each regardless of size,
    so tiles move in large batched triggers; xa prefetch depth (pool bufs)
    covers all 10 triggers so the stream never backpressures.
  - A few warmup matmuls run during the initial DMA wait to pre-ramp the
    PE clock (0.65/1.2 GHz p-states before 3us of continuous work).

Sharding: batch dim B=8, one batch per NeuronCore (data parallel).

Self-contained: hardcodes shapes from the problem spec
(x: [8, 4096, 512] f32; W*: [512, 512]; b*: [512]).
"""
import os
import sys

sys.path.insert(0, "/opt/trn_rl_repo")

import numpy as np
import ml_dtypes

import concourse.bass as bass
import concourse.mybir as mybir
import concourse.tile as tile
from concourse import bacc
from concourse.bass_utils import run_bass_kernel_spmd
from concourse.masks import make_identity
from concourse.tile_rust import add_dep_helper

B = 8          # batch -> one per core
N = 4096       # tokens per batch
D = 512        # model dim
NT = N // 128  # 32 row tiles
NSLAB = 8      # 512-token slabs for the out.T phase
N_CORES = 8

F32 = mybir.dt.float32
F32R = mybir.dt.float32r
BF16 = mybir.dt.bfloat16

# mode: "bf16" (bf16 x/P storage+matmul, f32r chain) or "f32r"
MODE = os.environ.get("GCF_MODE", "bf16")

# xa trigger batching: first single tile goes on gpsimd (earliest-ready
# queue), the rest stream on sync
XA_BATCHES = [1, 1, 2, 4, 4, 4, 4, 4, 4, 4]
N_WARM = 10

_built = {}


def _build(mode):
    if mode in _built:
        return _built[mode]

    big = BF16 if mode == "bf16" else F32R
    chain = F32R

    def mm_ap(ap, dt):
        return ap if ap.dtype == dt else ap.bitcast(dt)

    nc = bacc.Bacc("TRN2", target_bir_lowering=False, debug=False,
                   num_devices=N_CORES)

    # all DRAM tensors are laid out partition-dim first by the host
    xa_d = nc.dram_tensor("xa", (128, NT, D), big, kind="ExternalInput")
    xat_d = nc.dram_tensor("xat", (4, 128, NSLAB, D), big, kind="ExternalInput")
    khat_d = nc.dram_tensor("khat", (128, 4, D), chain, kind="ExternalInput")
    rhat_d = nc.dram_tensor("rhat", (128, 4, D), chain, kind="ExternalInput")
    # scl[:, 0:4] = v (bias), [:, 4:8] = sx columns, [:, 8:12] = khat col 512
    scl_d = nc.dram_tensor("scl", (128, 12), F32, kind="ExternalInput")
    # broadcast rows: [0] = Rhat row 512, [1] = m1row (both repl. 128x)
    brow_d = nc.dram_tensor("brow", (128, 2, D), F32, kind="ExternalInput")
    out_d = nc.dram_tensor("out", (4, 128, NSLAB, D), BF16,
                           kind="ExternalOutput")

    with tile.TileContext(nc) as tc:
        with (
            tc.tile_pool(name="xa", bufs=len(XA_BATCHES)) as xa_pool,
            tc.tile_pool(name="const", bufs=1) as const_pool,
            tc.tile_pool(name="gsb", bufs=1) as g_pool,
            tc.tile_pool(name="chain", bufs=1) as chain_pool,
            tc.tile_pool(name="outsb", bufs=3) as out_pool,
        ):
            # ---- xa stream on sync (first trigger fires earliest there) ----
            xa_tiles = []          # (tile, sub-index) per global row tile
            t0 = 0
            for bi, nb in enumerate(XA_BATCHES):
                xa_t = xa_pool.tile([128, 4, D], big, tag="xa")
                nc.sync.dma_start(xa_t[:, :nb, :], xa_d.ap()[:, t0:t0 + nb, :])
                for j in range(nb):
                    xa_tiles.append((xa_t, j))
                t0 += nb

            ident = const_pool.tile([128, 128], F32, tag="ident")
            make_identity(nc, ident[:])
            warm_sb = const_pool.tile([128, D], big, tag="warm")
            nc.vector.memset(warm_sb[:], 0.0)

            khat_sb = const_pool.tile([128, 4, D], chain, tag="khat")
            rhat_sb = const_pool.tile([128, 4, D], chain, tag="rhat")
            scl_sb = const_pool.tile([128, 12], F32, tag="scl")
            brow_sb = const_pool.tile([128, 2, D], F32, tag="brow")

            # ---- phase 1: G = x^T @ x; upper block-triangle only ----
            g_sb = [g_pool.tile([128, D], chain, tag=f"g{c}", name=f"g{c}")
                    for c in range(4)]
            with tc.tile_pool(name="psG", bufs=1, space="PSUM") as psG_pool:
                ps_ga = [psG_pool.tile([128, D - c * 128], F32, tag=f"ga{c}",
                                       name=f"ga{c}") for c in range(4)]
                # warmup matmuls: pre-ramp the PE clock while the first xa
                # tiles are still in flight (results unused; they borrow
                # ga0's bank, which G's first start=True resets anyway)
                for _ in range(N_WARM):
                    nc.tensor.matmul(ps_ga[0][:], mm_ap(warm_sb[:, :128], big),
                                     mm_ap(warm_sb[:], big),
                                     start=True, stop=True,
                                     skip_group_check=True)
                gate_mms = []
                for t in range(NT):
                    xa_t, j = xa_tiles[t]
                    for c in range(4):
                        mm = nc.tensor.matmul(
                            ps_ga[c][:],
                            mm_ap(xa_t[:, j, c * 128:(c + 1) * 128], big),
                            mm_ap(xa_t[:, j, c * 128:D], big),
                            start=(t == 0), stop=(t == NT - 1),
                        )
                        if c == 3:
                            gate_mms.append(mm)

                # consts on gpsimd, gated by first use so they never crowd
                # the xa stream: rhat feeds the first M1 group right at G
                # end; khat is first needed ~6us later at the P groups;
                # scl/brow at the chain adds
                for cd in (nc.gpsimd.dma_start(scl_sb[:], scl_d.ap()[:]),
                           nc.gpsimd.dma_start(brow_sb[:], brow_d.ap()[:])):
                    add_dep_helper(cd.ins, gate_mms[8].ins,
                                   reason="scl/brow (tiny, needed first) t=8")
                cd = nc.gpsimd.dma_start(rhat_sb[:], rhat_d.ap()[:])
                add_dep_helper(cd.ins, gate_mms[10].ins,
                               reason="rhat gated behind G t=10")
                cd = nc.gpsimd.dma_start(khat_sb[:], khat_d.ap()[:])
                add_dep_helper(cd.ins, gate_mms[20].ins,
                               reason="khat gated behind G t=20")

                # xat loads (sync queue, after the xa triggers in program
                # order) gated behind the G tail: during G the xa stream +
                # consts saturate HBM; the chain window is otherwise idle.
                xat_sb = [const_pool.tile([128, NSLAB, D], big, tag=f"xat{c}",
                                          name=f"xat{c}") for c in range(4)]
                for h in range(2):
                    for c in range(4):
                        xd = nc.sync.dma_start(
                            xat_sb[c][:, 4 * h:4 * h + 4, :],
                            xat_d.ap()[c][:, 4 * h:4 * h + 4, :])
                        add_dep_helper(xd.ins, gate_mms[26 if h == 0 else 31].ins,
                                       reason="xat gated behind G tail")

                # ---- phase 2 interleaved with G wrap-up; the upper-block
                # PSUM->SBUF copies are emitted just-in-time per column so
                # the DVE backlog never stalls the next M1 group ----
                # the copies run on the Scalar engine (idle until phase 3,
                # and it CAN read PSUM) so the DVE queue holds only the
                # chain adds — neither engine's backlog stalls the PE
                def copy_col(g1):
                    for c in range(g1 + 1):
                        nc.scalar.copy(
                            g_sb[c][:, g1 * 128:(g1 + 1) * 128],
                            ps_ga[c][:, (g1 - c) * 128:(g1 - c + 1) * 128])

                with tc.tile_pool(name="psC", bufs=2, space="PSUM") as psC_pool:
                    m1_sb = [chain_pool.tile([128, D], chain, tag=f"m1{c}",
                                             name=f"m1{c}") for c in range(4)]

                    def transpose_block(c1, c2):
                        # fill lower block (c2, c1) from upper (c1, c2)
                        ps_tr = psC_pool.tile([128, 128], F32, tag="tr", bufs=1)
                        nc.tensor.transpose(
                            ps_tr[:],
                            mm_ap(g_sb[c1][:, c2 * 128:(c2 + 1) * 128], F32),
                            ident[:],
                        )
                        # tr copies go on DVE (idle until the chain adds) so
                        # they are not queued behind scalar's column copies
                        nc.vector.tensor_copy(
                            g_sb[c2][:, c1 * 128:(c1 + 1) * 128], ps_tr[:])

                    # M1 groups g1 = 3..0; PE transposes run one group ahead
                    # of first use so their DVE copies are never on the
                    # critical path
                    for g1 in range(3, -1, -1):
                        copy_col(g1)
                        if g1 == 2:
                            transpose_block(2, 3)
                            transpose_block(1, 2)
                            transpose_block(1, 3)
                        elif g1 == 1:
                            transpose_block(0, 1)
                            transpose_block(0, 2)
                            transpose_block(0, 3)
                        ps = psC_pool.tile([128, D], F32, tag="chain", bufs=3)
                        for i, g2 in enumerate(
                                list(range(g1 + 1)) + list(range(g1 + 1, 4))):
                            nc.tensor.matmul(
                                ps[:],
                                mm_ap(g_sb[g2][:, g1 * 128:(g1 + 1) * 128],
                                      chain),
                                mm_ap(rhat_sb[:, g2, :], chain),
                                start=(i == 0), stop=(i == 3),
                            )
                        # m1 = ps + sx[g1-block] (x) Rhat[512,:] — the rank-1
                        # augmented term folds into the PSUM->SBUF copy
                        nc.vector.scalar_tensor_tensor(
                            m1_sb[g1][:], brow_sb[:, 0, :],
                            scl_sb[:, 4 + g1:5 + g1], ps[:],
                            mybir.AluOpType.mult, mybir.AluOpType.add)

                    p_sb = [chain_pool.tile([128, D], big, tag=f"p{c}",
                                            name=f"p{c}") for c in range(4)]
                    for g1 in range(4):
                        ps = psC_pool.tile([128, D], F32, tag="chain", bufs=3)
                        # g2 order 3..0: m1 adds complete in that order, so
                        # the group never waits on the most recent add
                        for i, g2 in enumerate(range(3, -1, -1)):
                            nc.tensor.matmul(
                                ps[:],
                                mm_ap(khat_sb[:, g2, g1 * 128:(g1 + 1) * 128],
                                      chain),
                                mm_ap(m1_sb[g2][:], chain),
                                start=(i == 0), stop=(i == 3),
                            )
                        # p = ps + Khat[g1-block, 512] (x) m1row
                        nc.vector.scalar_tensor_tensor(
                            p_sb[g1][:], brow_sb[:, 1, :],
                            scl_sb[:, 8 + g1:9 + g1], ps[:],
                            mybir.AluOpType.mult, mybir.AluOpType.add)

            # ---- phase 3: out.T[db,:] = sum_c P[c,db].T @ x.T[c,:] + v[db]
            # (+v fused into the Scalar-engine PSUM->SBUF copy as a
            # per-partition bias) ----
            with tc.tile_pool(name="psO", bufs=1, space="PSUM") as psO_pool:
                nst = 0
                for h in range(2):
                    for db in range(4):
                        ot = out_pool.tile([128, 4, D], BF16, tag="ot")
                        for si in range(4):
                            s = 4 * h + si
                            ps = psO_pool.tile([128, D], F32, tag="out", bufs=8)
                            for c in range(4):
                                nc.tensor.matmul(
                                    ps[:],
                                    mm_ap(p_sb[c][:, db * 128:(db + 1) * 128],
                                          big),
                                    mm_ap(xat_sb[c][:, s, :], big),
                                    start=(c == 0), stop=(c == 3),
                                )
                            nc.scalar.add(ot[:, si, :], ps[:],
                                          scl_sb[:, db:db + 1])
                            # split the final buffer's store (2+1+1 slabs) so
                            # the very last transfer is short
                            if h == 1 and db == 3 and si >= 1:
                                eng = nc.gpsimd if nst % 2 == 0 else nc.sync
                                nst += 1
                                lo, n = (0, 2) if si == 1 else (si, 1)
                                eng.dma_start(
                                    out_d.ap()[db][:, 4 * h + lo:4 * h + lo + n, :],
                                    ot[:, lo:lo + n, :])
                        if not (h == 1 and db == 3):
                            eng = nc.gpsimd if nst % 2 == 0 else nc.sync
                            nst += 1
                            eng.dma_start(
                                out_d.ap()[db][:, 4 * h:4 * h + 4, :], ot[:])

    nc.compile()
    _built[mode] = nc
    return nc


def _prep_host(x, Wq1_w, Wq1_b, Wq2_w, Wq2_b, WR_w, WR_b, mode):
    f, f8 = np.float32, np.float64
    W1a = np.concatenate([Wq1_w, Wq1_b[:, None]], axis=1)   # [512, 513]
    W2a = np.concatenate([Wq2_w, Wq2_b[:, None]], axis=1)
    WRa = np.concatenate([WR_w, WR_b[:, None]], axis=1)

    khatT = (W2a.T.astype(f8) @ W1a.astype(f8)).astype(f)   # [513, 513]
    rhat = WRa.T.astype(f)                                  # [513, 512]
    khat2 = np.ascontiguousarray(
        khatT[:D, :D].reshape(4, 128, D).transpose(1, 0, 2))
    rhat2 = np.ascontiguousarray(
        rhat[:D].reshape(4, 128, D).transpose(1, 0, 2))

    sx = x.sum(axis=1, dtype=f8).astype(f)                  # [B, 512]
    sxa = np.concatenate([sx, np.full((B, 1), float(N), f)], axis=1)
    m1row = (sxa.astype(f8) @ rhat.astype(f8)).astype(f)    # [B, 512]

    # v = P_aug[512,:] = Khat_aug[512,:] @ G_aug @ Rhat, host-computable in
    # O(N*D): z = x@k[:512] + k[512];  v = [x.T z | sum z] @ Rhat
    k = (W1a[:, D].astype(f8) @ W2a.astype(f8))             # [513]
    z = x.astype(f8) @ k[:D] + k[D]                         # [B, 4096]
    u = np.concatenate([np.einsum('bn,bnd->bd', z, x.astype(f8)),
                        z.sum(axis=1)[:, None]], axis=1)    # [B, 513]
    v = (u @ rhat.astype(f8)).astype(f)                     # [B, 512]

    # scl[:, 0:4] = v, [:, 4:8] = sx, [:, 8:12] = Khat[:, 512], col-major
    scl = np.concatenate([
        v.reshape(B, 4, 128), sx.reshape(B, 4, 128),
        np.broadcast_to(khatT[D, :D].reshape(1, 4, 128), (B, 4, 128)),
    ], axis=1).transpose(0, 2, 1).astype(f)                 # [B, 128, 12]
    scl = np.ascontiguousarray(scl)
    brow = np.stack([
        np.broadcast_to(rhat[D], (B, 128, D)),
        np.repeat(m1row[:, None, :], 128, axis=1),
    ], axis=2).astype(f)                                    # [B, 128, 2, D]
    brow = np.ascontiguousarray(brow)

    dt = ml_dtypes.bfloat16 if mode == "bf16" else f
    xa2 = np.ascontiguousarray(
        x.reshape(B, NT, 128, D).transpose(0, 2, 1, 3)).astype(dt)
    xat2 = np.ascontiguousarray(
        x.transpose(0, 2, 1).reshape(B, 4, 128, NSLAB, D)).astype(dt)

    return [
        {"xa": xa2[b], "xat": xat2[b], "khat": khat2, "rhat": rhat2,
         "scl": scl[b], "brow": brow[b]}
        for b in range(B)
    ]


def _post(res):
    # out.T tiles [db, p, slab, j] -> out[slab*512+j, db*128+p]
    return np.stack([
        np.ascontiguousarray(
            res.results[b]["out"].astype(np.float32)
            .transpose(2, 3, 0, 1)).reshape(N, D)
        for b in range(B)
    ])


def kernel(x, Wq1_w, Wq1_b, Wq2_w, Wq2_b, WR_w, WR_b):
    x = np.asarray(x, dtype=np.float32)
    args = [np.asarray(a, dtype=np.float32)
            for a in (Wq1_w, Wq1_b, Wq2_w, Wq2_b, WR_w, WR_b)]
    in_maps = _prep_host(x, *args, MODE)

    nc = _build(MODE)
    # the axon-tunneled device occasionally starts in a wedged state
    # (NRT_EXEC_UNIT_UNRECOVERABLE) and recovers on the next attempt
    last_err = None
    for attempt in range(3):
        try:
            res = run_bass_kernel_spmd(nc, in_maps, core_ids=list(range(N_CORES)))
            break
        except Exception as e:  # noqa: BLE001
            last_err = e
            import time as _time
            _time.sleep(2.0)
            try:
                import jax
                jax.clear_caches()
            except Exception:
                pass
    else:
        raise last_err
    return _post(res)


# revision 30
# speedup vs baseline: 1.1429x; 1.0004x over previous
"""Trainium2 Bass kernel for GCFAgg-style block:
    q1 = x@W1.T+b1; q2 = x@W2.T+b2; r = x@WR.T+br
    out = (q1 @ q2.T) @ r        (per batch, no softmax)

Key algebraic restructuring: with x_aug = [x | 1] and W*_aug = [W* | b*],
    out = x_aug @ (Khat @ (x_aug.T @ x_aug) @ Rhat)
where Khat = W1_aug.T @ W2_aug and Rhat = WR_aug.T are tiny host-precomputed
matrices. The device computes G = x.T @ x (symmetric: upper block-triangle
on PE, lower from PE transposes), the small chain P = Khat @ G @ Rhat, and
the projection out.T = P[:512].T @ x.T + v.

Work placement:
  - v (= P_aug row 512) is host-computed in O(N*D).
  - The rank-1 augmented terms of the chain (sx (x) rhat_row512 and
    khat_col512 (x) m1row) are materialized by the Scalar engine during the
    G window (scale-by-per-partition-scalar of a host-broadcast row) and
    folded into the chain's PSUM->SBUF copies as DVE adds — no K=1 PE
    matmuls.
  - G-symmetry: M1 groups run in order g1=3..0; group 3 needs only upper
    blocks, and each PE transpose that fills a lower block is interleaved
    right before the first group that consumes it.
  - out.T orientation makes +v a per-partition bias fused into the Scalar
    engine PSUM->SBUF copy; host reassembles the transposed output.

Perf notes (per core, PE @2.4GHz, ~332GB/s HBM):
  - PE ~127k cycles = 53us is the floor (G 41k, chain ~18k, out 65.5k).
  - bf16 x streams and P buy DMA bytes (bf16/f32r matmul are both
    1 cycle/row); the chain stays f32r for accuracy.
  - DMA triggers cost ~600ns of queue-engine time each regardless of size,
    so tiles move in large batched triggers; xa prefetch depth (pool bufs)
    covers all 10 triggers so the stream never backpressures.
  - A few warmup matmuls run during the initial DMA wait to pre-ramp the
    PE clock (0.65/1.2 GHz p-states before 3us of continuous work).

Sharding: batch dim B=8, one batch per NeuronCore (data parallel).

Self-contained: hardcodes shapes from the problem spec
(x: [8, 4096, 512] f32; W*: [512, 512]; b*: [512]).
"""
import os
import sys

sys.path.insert(0, "/opt/trn_rl_repo")

import numpy as np
import ml_dtypes

import concourse.bass as bass
import concourse.mybir as mybir
import concourse.tile as tile
from concourse import bacc
from concourse.bass_utils import run_bass_kernel_spmd
from concourse.masks import make_identity
from concourse.tile_rust import add_dep_helper

B = 8          # batch -> one per core
N = 4096       # tokens per batch
D = 512        # model dim
NT = N // 128  # 32 row tiles
NSLAB = 8      # 512-token slabs for the out.T phase
N_CORES = 8

F32 = mybir.dt.float32
F32R = mybir.dt.float32r
BF16 = mybir.dt.bfloat16

# mode: "bf16" (bf16 x/P storage+matmul, f32r chain) or "f32r"
MODE = os.environ.get("GCF_MODE", "bf16")

# xa trigger batching: first single tile goes on gpsimd (earliest-ready
# queue), the rest stream on sync
XA_BATCHES = [1, 1, 2, 2, 2, 4, 4, 4, 4, 4, 4]
N_WARM = 10

_built = {}


def _build(mode):
    if mode in _built:
        return _built[mode]

    big = BF16 if mode == "bf16" else F32R
    chain = F32R

    def mm_ap(ap, dt):
        return ap if ap.dtype == dt else ap.bitcast(dt)

    nc = bacc.Bacc("TRN2", target_bir_lowering=False, debug=False,
                   num_devices=N_CORES)

    # all DRAM tensors are laid out partition-dim first by the host
    xa_d = nc.dram_tensor("xa", (128, NT, D), big, kind="ExternalInput")
    xat_d = nc.dram_tensor("xat", (4, 128, NSLAB, D), big, kind="ExternalInput")
    khat_d = nc.dram_tensor("khat", (128, 4, D), chain, kind="ExternalInput")
    rhat_d = nc.dram_tensor("rhat", (128, 4, D), chain, kind="ExternalInput")
    # scl[:, 0:4] = v (bias), [:, 4:8] = sx columns, [:, 8:12] = khat col 512
    scl_d = nc.dram_tensor("scl", (128, 12), F32, kind="ExternalInput")
    # broadcast rows: [0] = Rhat row 512, [1] = m1row (both repl. 128x)
    brow_d = nc.dram_tensor("brow", (128, 2, D), F32, kind="ExternalInput")
    out_d = nc.dram_tensor("out", (4, 128, NSLAB, D), BF16,
                           kind="ExternalOutput")

    with tile.TileContext(nc) as tc:
        with (
            tc.tile_pool(name="xa", bufs=len(XA_BATCHES)) as xa_pool,
            tc.tile_pool(name="const", bufs=1) as const_pool,
            tc.tile_pool(name="gsb", bufs=1) as g_pool,
            tc.tile_pool(name="chain", bufs=1) as chain_pool,
            tc.tile_pool(name="outsb", bufs=3) as out_pool,
        ):
            # ---- xa stream on sync (first trigger fires earliest there) ----
            xa_tiles = []          # (tile, sub-index) per global row tile
            t0 = 0
            for bi, nb in enumerate(XA_BATCHES):
                xa_t = xa_pool.tile([128, 4, D], big, tag="xa")
                nc.sync.dma_start(xa_t[:, :nb, :], xa_d.ap()[:, t0:t0 + nb, :])
                for j in range(nb):
                    xa_tiles.append((xa_t, j))
                t0 += nb

            ident = const_pool.tile([128, 128], F32, tag="ident")
            make_identity(nc, ident[:])
            warm_sb = const_pool.tile([128, D], big, tag="warm")
            nc.vector.memset(warm_sb[:], 0.0)

            khat_sb = const_pool.tile([128, 4, D], chain, tag="khat")
            rhat_sb = const_pool.tile([128, 4, D], chain, tag="rhat")
            scl_sb = const_pool.tile([128, 12], F32, tag="scl")
            brow_sb = const_pool.tile([128, 2, D], F32, tag="brow")

            # ---- phase 1: G = x^T @ x; upper block-triangle only ----
            g_sb = [g_pool.tile([128, D], chain, tag=f"g{c}", name=f"g{c}")
                    for c in range(4)]
            with tc.tile_pool(name="psG", bufs=1, space="PSUM") as psG_pool:
                ps_ga = [psG_pool.tile([128, D - c * 128], F32, tag=f"ga{c}",
                                       name=f"ga{c}") for c in range(4)]
                # warmup matmuls: pre-ramp the PE clock while the first xa
                # tiles are still in flight (results unused; they borrow
                # ga0's bank, which G's first start=True resets anyway)
                for _ in range(N_WARM):
                    nc.tensor.matmul(ps_ga[0][:], mm_ap(warm_sb[:, :128], big),
                                     mm_ap(warm_sb[:], big),
                                     start=True, stop=True,
                                     skip_group_check=True)
                gate_mms = []
                for t in range(NT):
                    xa_t, j = xa_tiles[t]
                    for c in range(4):
                        mm = nc.tensor.matmul(
                            ps_ga[c][:],
                            mm_ap(xa_t[:, j, c * 128:(c + 1) * 128], big),
                            mm_ap(xa_t[:, j, c * 128:D], big),
                            start=(t == 0), stop=(t == NT - 1),
                        )
                        if c == 3:
                            gate_mms.append(mm)

                # consts on gpsimd, gated by first use so they never crowd
                # the xa stream: rhat feeds the first M1 group right at G
                # end; khat is first needed ~6us later at the P groups;
                # scl/brow at the chain adds
                for cd in (nc.gpsimd.dma_start(scl_sb[:], scl_d.ap()[:]),
                           nc.gpsimd.dma_start(brow_sb[:], brow_d.ap()[:])):
                    add_dep_helper(cd.ins, gate_mms[8].ins,
                                   reason="scl/brow (tiny, needed first) t=8")
                cd = nc.gpsimd.dma_start(rhat_sb[:], rhat_d.ap()[:])
                add_dep_helper(cd.ins, gate_mms[10].ins,
                               reason="rhat gated behind G t=10")
                cd = nc.gpsimd.dma_start(khat_sb[:], khat_d.ap()[:])
                add_dep_helper(cd.ins, gate_mms[20].ins,
                               reason="khat gated behind G t=20")

                # xat loads (sync queue, after the xa triggers in program
                # order) gated behind the G tail: during G the xa stream +
                # consts saturate HBM; the chain window is otherwise idle.
                xat_sb = [const_pool.tile([128, NSLAB, D], big, tag=f"xat{c}",
                                          name=f"xat{c}") for c in range(4)]
                for h in range(2):
                    for c in range(4):
                        xd = nc.sync.dma_start(
                            xat_sb[c][:, 4 * h:4 * h + 4, :],
                            xat_d.ap()[c][:, 4 * h:4 * h + 4, :])
                        add_dep_helper(xd.ins, gate_mms[26 if h == 0 else 31].ins,
                                       reason="xat gated behind G tail")

                # ---- phase 2 interleaved with G wrap-up; the upper-block
                # PSUM->SBUF copies are emitted just-in-time per column so
                # the DVE backlog never stalls the next M1 group ----
                # the copies run on the Scalar engine (idle until phase 3,
                # and it CAN read PSUM) so the DVE queue holds only the
                # chain adds — neither engine's backlog stalls the PE
                def copy_col(g1):
                    for c in range(g1 + 1):
                        nc.scalar.copy(
                            g_sb[c][:, g1 * 128:(g1 + 1) * 128],
                            ps_ga[c][:, (g1 - c) * 128:(g1 - c + 1) * 128])

                with tc.tile_pool(name="psC", bufs=2, space="PSUM") as psC_pool:
                    m1_sb = [chain_pool.tile([128, D], chain, tag=f"m1{c}",
                                             name=f"m1{c}") for c in range(4)]

                    def transpose_block(c1, c2):
                        # fill lower block (c2, c1) from upper (c1, c2)
                        ps_tr = psC_pool.tile([128, 128], F32, tag="tr", bufs=1)
                        nc.tensor.transpose(
                            ps_tr[:],
                            mm_ap(g_sb[c1][:, c2 * 128:(c2 + 1) * 128], F32),
                            ident[:],
                        )
                        # tr copies go on DVE (idle until the chain adds) so
                        # they are not queued behind scalar's column copies
                        nc.vector.tensor_copy(
                            g_sb[c2][:, c1 * 128:(c1 + 1) * 128], ps_tr[:])

                    # M1 groups g1 = 3..0; PE transposes run one group ahead
                    # of first use so their DVE copies are never on the
                    # critical path
                    for g1 in range(3, -1, -1):
                        copy_col(g1)
                        if g1 == 2:
                            transpose_block(2, 3)
                            transpose_block(1, 2)
                            transpose_block(1, 3)
                        elif g1 == 1:
                            transpose_block(0, 1)
                            transpose_block(0, 2)
                            transpose_block(0, 3)
                        ps = psC_pool.tile([128, D], F32, tag="chain", bufs=3)
                        for i, g2 in enumerate(
                                list(range(g1 + 1)) + list(range(g1 + 1, 4))):
                            nc.tensor.matmul(
                                ps[:],
                                mm_ap(g_sb[g2][:, g1 * 128:(g1 + 1) * 128],
                                      chain),
                                mm_ap(rhat_sb[:, g2, :], chain),
                                start=(i == 0), stop=(i == 3),
                            )
                        # m1 = ps + sx[g1-block] (x) Rhat[512,:] — the rank-1
                        # augmented term folds into the PSUM->SBUF copy
                        nc.vector.scalar_tensor_tensor(
                            m1_sb[g1][:], brow_sb[:, 0, :],
                            scl_sb[:, 4 + g1:5 + g1], ps[:],
                            mybir.AluOpType.mult, mybir.AluOpType.add)

                    p_sb = [chain_pool.tile([128, D], big, tag=f"p{c}",
                                            name=f"p{c}") for c in range(4)]
                    for g1 in range(4):
                        ps = psC_pool.tile([128, D], F32, tag="chain", bufs=3)
                        # g2 order 3..0: m1 adds complete in that order, so
                        # the group never waits on the most recent add
                        for i, g2 in enumerate(range(3, -1, -1)):
                            nc.tensor.matmul(
                                ps[:],
                                mm_ap(khat_sb[:, g2, g1 * 128:(g1 + 1) * 128],
                                      chain),
                                mm_ap(m1_sb[g2][:], chain),
                                start=(i == 0), stop=(i == 3),
                            )
                        # p = ps + Khat[g1-block, 512] (x) m1row
                        nc.vector.scalar_tensor_tensor(
                            p_sb[g1][:], brow_sb[:, 1, :],
                            scl_sb[:, 8 + g1:9 + g1], ps[:],
                            mybir.AluOpType.mult, mybir.AluOpType.add)

            # ---- phase 3: out.T[db,:] = sum_c P[c,db].T @ x.T[c,:] + v[db]
            # (+v fused into the Scalar-engine PSUM->SBUF copy as a
            # per-partition bias) ----
            with tc.tile_pool(name="psO", bufs=1, space="PSUM") as psO_pool:
                nst = 0
                for h in range(2):
                    for db in range(4):
                        ot = out_pool.tile([128, 4, D], BF16, tag="ot")
                        for si in range(4):
                            s = 4 * h + si
                            ps = psO_pool.tile([128, D], F32, tag="out", bufs=8)
                            for c in range(4):
                                nc.tensor.matmul(
                                    ps[:],
                                    mm_ap(p_sb[c][:, db * 128:(db + 1) * 128],
                                          big),
                                    mm_ap(xat_sb[c][:, s, :], big),
                                    start=(c == 0), stop=(c == 3),
                                )
                            nc.scalar.add(ot[:, si, :], ps[:],
                                          scl_sb[:, db:db + 1])
                            # split the final buffer's store (2+1+1 slabs) so
                            # the very last transfer is short
                            if h == 1 and db == 3 and si >= 1:
                                eng = nc.sync
                                nst += 1
                                lo, n = (0, 2) if si == 1 else (si, 1)
                                eng.dma_start(
                                    out_d.ap()[db][:, 4 * h + lo:4 * h + lo + n, :],
                                    ot[:, lo:lo + n, :])
                        if not (h == 1 and db == 3):
                            eng = nc.sync
                            nst += 1
                            eng.dma_start(
                                out_d.ap()[db][:, 4 * h:4 * h + 4, :], ot[:])

    nc.compile()
    _built[mode] = nc
    return nc


def _prep_host(x, Wq1_w, Wq1_b, Wq2_w, Wq2_b, WR_w, WR_b, mode):
    f, f8 = np.float32, np.float64
    W1a = np.concatenate([Wq1_w, Wq1_b[:, None]], axis=1)   # [512, 513]
    W2a = np.concatenate([Wq2_w, Wq2_b[:, None]], axis=1)
    WRa = np.concatenate([WR_w, WR_b[:, None]], axis=1)

    khatT = (W2a.T.astype(f8) @ W1a.astype(f8)).astype(f)   # [513, 513]
    rhat = WRa.T.astype(f)                                  # [513, 512]
    khat2 = np.ascontiguousarray(
        khatT[:D, :D].reshape(4, 128, D).transpose(1, 0, 2))
    rhat2 = np.ascontiguousarray(
        rhat[:D].reshape(4, 128, D).transpose(1, 0, 2))

    sx = x.sum(axis=1, dtype=f8).astype(f)                  # [B, 512]
    sxa = np.concatenate([sx, np.full((B, 1), float(N), f)], axis=1)
    m1row = (sxa.astype(f8) @ rhat.astype(f8)).astype(f)    # [B, 512]

    # v = P_aug[512,:] = Khat_aug[512,:] @ G_aug @ Rhat, host-computable in
    # O(N*D): z = x@k[:512] + k[512];  v = [x.T z | sum z] @ Rhat
    k = (W1a[:, D].astype(f8) @ W2a.astype(f8))             # [513]
    z = x.astype(f8) @ k[:D] + k[D]                         # [B, 4096]
    u = np.concatenate([np.einsum('bn,bnd->bd', z, x.astype(f8)),
                        z.sum(axis=1)[:, None]], axis=1)    # [B, 513]
    v = (u @ rhat.astype(f8)).astype(f)                     # [B, 512]

    # scl[:, 0:4] = v, [:, 4:8] = sx, [:, 8:12] = Khat[:, 512], col-major
    scl = np.concatenate([
        v.reshape(B, 4, 128), sx.reshape(B, 4, 128),
        np.broadcast_to(khatT[D, :D].reshape(1, 4, 128), (B, 4, 128)),
    ], axis=1).transpose(0, 2, 1).astype(f)                 # [B, 128, 12]
    scl = np.ascontiguousarray(scl)
    brow = np.stack([
        np.broadcast_to(rhat[D], (B, 128, D)),
        np.repeat(m1row[:, None, :], 128, axis=1),
    ], axis=2).astype(f)                                    # [B, 128, 2, D]
    brow = np.ascontiguousarray(brow)

    dt = ml_dtypes.bfloat16 if mode == "bf16" else f
    xa2 = np.ascontiguousarray(
        x.reshape(B, NT, 128, D).transpose(0, 2, 1, 3)).astype(dt)
    xat2 = np.ascontiguousarray(
        x.transpose(0, 2, 1).reshape(B, 4, 128, NSLAB, D)).astype(dt)

    return [
        {"xa": xa2[b], "xat": xat2[b], "khat": khat2, "rhat": rhat2,
         "scl": scl[b], "brow": brow[b]}
        for b in range(B)
    ]


def _post(res):
    # out.T tiles [db, p, slab, j] -> out[slab*512+j, db*128+p]
    return np.stack([
        np.ascontiguousarray(
            res.results[b]["out"].astype(np.float32)
            .transpose(2, 3, 0, 1)).reshape(N, D)
        for b in range(B)
    ])


def kernel(x, Wq1_w, Wq1_b, Wq2_w, Wq2_b, WR_w, WR_b):
    x = np.asarray(x, dtype=np.float32)
    args = [np.asarray(a, dtype=np.float32)
            for a in (Wq1_w, Wq1_b, Wq2_w, Wq2_b, WR_w, WR_b)]
    in_maps = _prep_host(x, *args, MODE)

    nc = _build(MODE)
    # the axon-tunneled device occasionally starts in a wedged state
    # (NRT_EXEC_UNIT_UNRECOVERABLE) and recovers on the next attempt
    last_err = None
    for attempt in range(3):
        try:
            res = run_bass_kernel_spmd(nc, in_maps, core_ids=list(range(N_CORES)))
            break
        except Exception as e:  # noqa: BLE001
            last_err = e
            import time as _time
            _time.sleep(2.0)
            try:
                import jax
                jax.clear_caches()
            except Exception:
                pass
    else:
        raise last_err
    return _post(res)
